# revision 42
# baseline (speedup 1.0000x reference)
"""Trainium2 Bass kernel for nn_GAT_KDE_14766097563859.

2-layer GAT over a 2048-node graph + per-(graph,layer) KDE soft-quantile
readouts. SPMD over 8 NeuronCores: GAT replicated, KDE sharded by feature dim
(each core owns 16 of 128 dims for all 12 (layer,graph) units = 192 slots).

v3 (over the v2 baseline):
- density evaluated on a 125-point coarse grid; cumsum+linear-interp back to
  the 500-point cdf is folded into one constant [125,500] matmul matrix
  (linear maps compose), cutting exp/matmul volume 4x
- per-slot node-sum fused into the exp activation via accum_out (zero DVE)
- quantile stage flipped to [fine-grid part, slot free] orientation:
  sigma(-100|d|) = min(sigma(-100d), sigma(100d)) with the q-shift folded
  into the sigmoid bias; S0/T1 reductions over the grid run on PE
- PSUM->SBUF copies moved to Act; xh_s/xsq on Act (scale-AP / Square, with
  the -0.5 folded into the repx constant); per-graph xs kept, not re-derived
"""
import os
import sys
sys.path.insert(0, "/opt/trn_rl_repo")
import numpy as np

import concourse.bass as bass
import concourse.bacc as bacc
import concourse.mybir as mybir
from concourse.tile import TileContext
from concourse.masks import make_identity
from concourse import library_config

F32 = mybir.dt.float32
F32R = mybir.dt.float32r
BF16 = mybir.dt.bfloat16
I32 = mybir.dt.int32
I16 = mybir.dt.int16
AF = mybir.ActivationFunctionType
ALU = mybir.AluOpType
AX = mybir.AxisListType

G, NG, N, E = 4, 512, 2048, 32768
IN_DIM, HID, HEADS, HC, OUT_DIM = 128, 32, 4, 128, 32
N_LAYERS, GRID, Q, NEG_SLOPE = 2, 500, 20, 0.2
N_CORES = 8
UNITS = (N_LAYERS + 1) * G            # 12, unit u = l*G + g
DPC = 16                              # dims per core
SLOTS = UNITS * DPC                   # 192
GEFF = 500                            # fine grid (matches reference GRID)
GC = 125                              # coarse density grid
GCP = 128                             # GC padded (fp32r matmul needs free%4==0)
BATCH = 8                             # chunks per gather batch (1024 idxs = SWDGE ring cap)
TBLW = 256                            # bf16 cols of xh_al row (512B, dma_gather needs %256B)
ALDW = 128                            # bf16 cols of al_d row (256B)


def _edge_prep(edge_index):
    src = edge_index[0].astype(np.int64)
    dst = edge_index[1].astype(np.int64)
    s_all = np.concatenate([src, np.arange(N)])
    d_all = np.concatenate([dst, np.arange(N)])
    order = np.argsort(d_all, kind="stable")
    s_s, d_s = s_all[order], d_all[order]
    cs, cd, ct, cm = [], [], [], []
    for t in range(16):
        sel = (d_s // 128) == t
        se, de = s_s[sel], d_s[sel]
        ne = len(se)
        npad = (-ne) % 128
        se = np.concatenate([se, np.zeros(npad, np.int64)])
        de = np.concatenate([de, np.full(npad, t * 128, np.int64)])
        rm = np.concatenate([np.ones(ne, bool), np.zeros(npad, bool)])
        for c0 in range(0, len(se), 128):
            cs.append(se[c0:c0 + 128]); cd.append(de[c0:c0 + 128])
            cm.append(rm[c0:c0 + 128]); ct.append(t)
    C = len(cs)
    s_idx = np.stack(cs, 1).astype(np.int32)                  # [128, C] src ids
    d_idx = np.stack(cd, 1).astype(np.int32)                  # [128, C] dst ids
    dlf = (np.stack(cd, 1) - np.asarray(ct)[None, :] * 128).astype(np.float32)
    maskneg = np.stack(cm, 1).astype(np.float32)  # 1 valid, 0 pad
    return s_idx, d_idx, dlf, maskneg, np.asarray(ct, np.int32)


def build_program(C, chunk_tile, reps=1):
    nc = bacc.Bacc(None, target_bir_lowering=False, debug=True)

    xT_in = nc.declare_dram_parameter("xT", [128, N], F32, isOutput=False)
    Wi = [nc.declare_dram_parameter(f"W{l}", [128, HC], F32, isOutput=False) for l in range(2)]
    Avi = [nc.declare_dram_parameter(f"Av{l}", [128, 8], F32, isOutput=False) for l in range(2)]
    bci = [nc.declare_dram_parameter(f"bcol{l}", [128, 1], F32, isOutput=False) for l in range(2)]
    sel_in = nc.declare_dram_parameter("sel", [128, DPC], F32, isOutput=False)
    iota3_in = nc.declare_dram_parameter("iota3", [3, GCP], F32R, isOutput=False)
    triM_in = nc.declare_dram_parameter("triM", [GC, GEFF], F32, isOutput=False)
    oiT_in = nc.declare_dram_parameter("oiT", [128, 2, 4], F32, isOutput=False)
    qbias_in = nc.declare_dram_parameter("qbias", [128, 2 * Q + 1], F32, isOutput=False)
    iotaF_in = nc.declare_dram_parameter("iotaF", [128, 128], BF16, isOutput=False)
    CW = ((C + BATCH - 1) // BATCH) * BATCH * 8
    sidx_in = nc.declare_dram_parameter("sidx", [128, CW], I16, isOutput=False)
    didx_in = nc.declare_dram_parameter("didx", [128, CW], I16, isOutput=False)
    dlfb_in = nc.declare_dram_parameter("dlfb", [128, C], BF16, isOutput=False)
    mneg_in = nc.declare_dram_parameter("mneg", [128, C], F32, isOutput=False)
    repq_in = nc.declare_dram_parameter("repq", [DPC, 3, 512], F32R, isOutput=False)
    repx_in = nc.declare_dram_parameter("repx", [DPC, 4, 3, 128], F32R, isOutput=False)

    s0t1_out = nc.declare_dram_parameter("s0t1", [2 * Q, SLOTS], F32, isOutput=True)
    mndl_out = nc.declare_dram_parameter("mndl2", [DPC, 2 * UNITS], F32, isOutput=True)
    pmean_out = nc.declare_dram_parameter("pmean", [128, UNITS], F32, isOutput=True)
    pmax_out = nc.declare_dram_parameter("pmax", [128, UNITS], F32, isOutput=True)
    KDBG = os.environ.get("KDEBUG") == "1"
    if KDBG:
        dbg_dens = nc.declare_dram_parameter("dbg_dens", [GC, SLOTS], F32, isOutput=True)
        dbg_cdfn = nc.declare_dram_parameter("dbg_cdfn", [128, 4 * SLOTS], F32, isOutput=True)

    xh_al_hbm = nc.dram_tensor("xh_al", [N, TBLW], BF16)
    ald_hbm = nc.dram_tensor("ald", [N, ALDW], BF16)

    with TileContext(nc) as tc:
        with (
            tc.tile_pool(name="cst", bufs=1) as cst,
            tc.tile_pool(name="wrk", bufs=2) as sb,
            tc.tile_pool(name="ps", bufs=1, space="PSUM") as ps,
        ):
            # ---------------- resident state ----------------
            curT = [cst.tile([128, N], F32, tag=f"curT{l}", name=f"curT{l}") for l in range(3)]
            W = [cst.tile([128, HC], F32, tag=f"Wt{l}", name=f"Wt{l}") for l in range(2)]
            Av = [cst.tile([128, 8], F32, tag=f"Avt{l}", name=f"Avt{l}") for l in range(2)]
            bcol = [cst.tile([128, 1], F32, tag=f"bct{l}", name=f"bct{l}") for l in range(2)]
            sel = cst.tile([128, DPC], F32)
            iota3 = cst.tile([3, GCP], F32R)
            triM = cst.tile([GC, GEFF], F32)
            oiT = cst.tile([128, 2, 4], F32)
            qbias = cst.tile([128, 2 * Q + 1], F32)
            iotaF = cst.tile([128, 128], BF16)
            sidx = cst.tile([128, CW], I16)
            didx = cst.tile([128, CW], I16)
            dlfb = cst.tile([128, C], BF16)
            mneg = cst.tile([128, C], F32)
            repq = cst.tile([DPC, 3, 512], F32R)
            repx = cst.tile([DPC, 4, 3, 128], F32R)
            ones16 = cst.tile([DPC, 512], F32R)
            onesrow = cst.tile([1, 128], F32)
            densC = cst.tile([GC, SLOTS], F32)
            pmean = cst.tile([128, UNITS], F32)
            pmax = cst.tile([128, UNITS], F32)
            id128 = cst.tile([128, 128], F32)
            xhT = cst.tile([128, N], F32)
            al = cst.tile([128, 16, 8], F32)
            stage = cst.tile([128, 16, TBLW], BF16)
            stage_d = cst.tile([128, 16, ALDW], BF16)
            cdfn = cst.tile([128, 4, SLOTS], F32)
            nc.gpsimd.load_library(library_config.mlp)
            make_identity(nc, id128[:])
            nc.gpsimd.dma_start(out=curT[0][:], in_=xT_in[:])
            for l in range(2):
                nc.gpsimd.dma_start(out=W[l][:], in_=Wi[l][:])
                nc.gpsimd.dma_start(out=Av[l][:], in_=Avi[l][:])
                nc.gpsimd.dma_start(out=bcol[l][:], in_=bci[l][:])
            nc.gpsimd.dma_start(out=sel[:], in_=sel_in[:])
            nc.gpsimd.dma_start(out=iota3[:], in_=iota3_in[:])
            nc.gpsimd.dma_start(out=triM[:], in_=triM_in[:])
            nc.gpsimd.dma_start(out=oiT[:], in_=oiT_in[:])
            nc.gpsimd.dma_start(out=qbias[:], in_=qbias_in[:])
            nc.gpsimd.dma_start(out=iotaF[:], in_=iotaF_in[:])
            nc.gpsimd.dma_start(out=sidx[:], in_=sidx_in[:])
            nc.gpsimd.dma_start(out=didx[:], in_=didx_in[:])
            nc.gpsimd.dma_start(out=dlfb[:], in_=dlfb_in[:])
            nc.gpsimd.dma_start(out=mneg[:], in_=mneg_in[:])
            nc.gpsimd.dma_start(out=repq[:], in_=repq_in[:])
            nc.gpsimd.dma_start(out=repx[:], in_=repx_in[:])
            nc.gpsimd.memset(ones16[:].bitcast(F32), 1.0)
            nc.gpsimd.memset(onesrow[:], 1.0)
            nc.gpsimd.memset(stage[:].bitcast(mybir.dt.uint16), 0)
            nc.gpsimd.memset(stage_d[:].bitcast(mybir.dt.uint16), 0)
            # constant 1.0 columns of the gather table (col 33h+32), set once
            nc.gpsimd.memset(stage[:, :, 0:132].rearrange("p t (h c) -> p t h c", c=33)[:, :, :, 32:33].bitcast(mybir.dt.uint16), 0x3F80)

            # ---------------- per-layer stats (all 4 graphs) ----------------
            def stats_phase(l):
                cur = curT[l]
                xsall = sb.tile([DPC, G, 512], F32, tag="xsall", bufs=2)
                per_g = []
                for g in range(G):
                    u = l * G + g
                    blk = cur[:, 512 * g:512 * (g + 1)]
                    nc.vector.tensor_reduce(out=pmean[:, u:u + 1], in_=blk, axis=AX.X, op=ALU.add)
                    nc.vector.tensor_reduce(out=pmax[:, u:u + 1], in_=blk, axis=AX.X, op=ALU.max)
                    ps_xs = ps.tile([DPC, 512], F32, tag="ps_m", bufs=2)
                    nc.tensor.matmul(ps_xs[:], sel[:], blk, start=True, stop=True)
                    xs = xsall[:, g, :]
                    nc.scalar.activation(xs, ps_xs[:], AF.Identity)
                    mn = sb.tile([DPC, 1], F32, tag=f"mn{g}", name=f"mn{g}", bufs=2)
                    mx = sb.tile([DPC, 1], F32, tag="mx")
                    sm = sb.tile([DPC, 1], F32, tag="sm")
                    sq = sb.tile([DPC, 1], F32, tag="sq")
                    nc.vector.tensor_reduce(out=mn[:], in_=xs, axis=AX.X, op=ALU.min)
                    nc.vector.tensor_reduce(out=mx[:], in_=xs, axis=AX.X, op=ALU.max)
                    nc.vector.tensor_reduce(out=sm[:], in_=xs, axis=AX.X, op=ALU.add)
                    xsq0 = sb.tile([DPC, 512], F32, tag="xsq0")
                    nc.vector.tensor_tensor(out=xsq0[:], in0=xs, in1=xs, op=ALU.mult)
                    nc.vector.tensor_reduce(out=sq[:], in_=xsq0[:], axis=AX.X, op=ALU.add)
                    var = sb.tile([DPC, 1], F32, tag=f"var{g}", name=f"var{g}", bufs=2)
                    mean = sb.tile([DPC, 1], F32, tag="mean")
                    nc.vector.tensor_scalar(out=mean[:], in0=sm[:], scalar1=1.0 / 512,
                                            scalar2=None, op0=ALU.mult)
                    nc.vector.tensor_scalar(out=var[:], in0=sq[:], scalar1=1.0 / 512,
                                            scalar2=None, op0=ALU.mult)
                    m2 = sb.tile([DPC, 1], F32, tag="m2")
                    nc.vector.tensor_tensor(out=m2[:], in0=mean[:], in1=mean[:], op=ALU.mult)
                    nc.vector.tensor_tensor(out=var[:], in0=var[:], in1=m2[:], op=ALU.subtract)
                    nc.vector.tensor_scalar(out=mn[:], in0=mn[:], scalar1=-1e-6, scalar2=None, op0=ALU.add)
                    nc.vector.tensor_scalar(out=mx[:], in0=mx[:], scalar1=1e-6, scalar2=None, op0=ALU.add)
                    dl = sb.tile([DPC, 1], F32, tag=f"dl{g}", name=f"dl{g}", bufs=2)
                    nc.vector.tensor_tensor(out=dl[:], in0=mx[:], in1=mn[:], op=ALU.subtract)
                    nc.vector.tensor_scalar(out=dl[:], in0=dl[:], scalar1=1.0 / (GEFF - 1), scalar2=None, op0=ALU.mult)
                    per_g.append((mn, dl, var))
                # batched sqrt on Act (one table region), mn/dl -> row layout
                out = []
                for g, (mn, dl, var) in enumerate(per_g):
                    u = l * G + g
                    std = sb.tile([DPC, 1], F32, tag=f"std{g}", name=f"std{g}", bufs=2)
                    nc.scalar.activation(std[:], var[:], AF.Sqrt)
                    nc.sync.dma_start(out=mndl_out[:, 2 * u:2 * u + 1], in_=mn[:])
                    nc.sync.dma_start(out=mndl_out[:, 2 * u + 1:2 * u + 2], in_=dl[:])
                    out.append((mn, dl, std))
                return out, xsall

            def derive_graph(l, g, xsall, mn, dl, std):
                """Per-graph slot-pass prep: build cstack + X rows from kept xs."""
                xs = xsall[:, g, :]
                h = sb.tile([DPC, 1], F32, tag="h")
                nc.vector.tensor_scalar(out=h[:], in0=std[:], scalar1=float(1e-8 / 3),
                                        scalar2=float(1.06 * 512 ** -0.2),
                                        op0=ALU.add, op1=ALU.mult)
                rh = sb.tile([DPC, 1], F32, tag="rh", bufs=2)
                nc.vector.reciprocal(out=rh[:], in_=h[:])
                # recentered basis: k' = k-(GC-1)/2, x' = (x-c)/h with c the
                # grid midpoint -> a0 = 0 and all poly terms stay O(span/2h),
                # minimizing f32r (tf32) rounding of the stored poly values
                a0 = sb.tile([DPC, 1], F32, tag="a0")
                a1 = sb.tile([DPC, 1], F32, tag="a1")
                nc.gpsimd.memset(a0[:], 0.0)
                # coarse grid step is (GEFF-1)/(GC-1) fine steps
                nc.vector.scalar_tensor_tensor(
                    out=a1[:], in0=dl[:], scalar=float((GEFF - 1) / (GC - 1)),
                    in1=rh[:], op0=ALU.mult, op1=ALU.mult)
                mnrh = sb.tile([DPC, 1], F32, tag="mnrh")
                nc.vector.tensor_tensor(out=mnrh[:], in0=mn[:], in1=rh[:], op=ALU.mult)
                biasc = sb.tile([DPC, 1], F32, tag="biasc", bufs=2)
                nc.vector.scalar_tensor_tensor(
                    out=biasc[:], in0=a1[:], scalar=-float((GC - 1) / 2),
                    in1=mnrh[:], op0=ALU.mult, op1=ALU.subtract)
                # cstack [16, 9]: r0=(q0,q1,q2) r1=(a0,a1,0) r2=(1,0,0)
                cstack = sb.tile([DPC, 9], F32R, tag="cs", bufs=2)
                nc.gpsimd.memset(cstack[:].bitcast(F32), 0.0)
                nc.vector.tensor_tensor(out=cstack[:, 0:1], in0=a0[:], in1=a0[:], op=ALU.mult)
                nc.vector.tensor_scalar(out=cstack[:, 0:1], in0=cstack[:, 0:1], scalar1=-0.5, scalar2=None, op0=ALU.mult)
                nc.vector.tensor_tensor(out=cstack[:, 1:2], in0=a0[:], in1=a1[:], op=ALU.mult)
                nc.vector.tensor_scalar(out=cstack[:, 1:2], in0=cstack[:, 1:2], scalar1=-1.0, scalar2=None, op0=ALU.mult)
                nc.vector.tensor_tensor(out=cstack[:, 2:3], in0=a1[:], in1=a1[:], op=ALU.mult)
                nc.vector.tensor_scalar(out=cstack[:, 2:3], in0=cstack[:, 2:3], scalar1=-0.5, scalar2=None, op0=ALU.mult)
                nc.vector.tensor_copy(cstack[:, 3:4], a0[:])
                nc.vector.tensor_copy(cstack[:, 4:5], a1[:])
                nc.gpsimd.memset(cstack[:, 6:7].bitcast(F32), 1.0)
                xh_s = sb.tile([DPC, 512], F32R, tag="xh_s", bufs=2)
                nc.scalar.activation(xh_s[:], xs, AF.Identity, scale=rh[:, 0:1],
                                     bias=biasc[:, 0:1])
                xsq = sb.tile([DPC, 512], F32R, tag="xsq", bufs=2)
                nc.scalar.activation(xsq[:], xh_s[:], AF.Square)
                return cstack, xh_s, xsq

            # ---------------- per-(unit) prep: Lq / X4 ----------------
            def unit_prep(l, g, cstack_g, xh_s_g, xsq_g):
                # lh [3, 512]: col 128m+32s+r' = cstack triple r' of slot k=4m+s
                ps_lh = ps.tile([3, 512], F32, tag="ps_m", bufs=2)
                for r in range(3):
                    nc.tensor.matmul(ps_lh[:], cstack_g[:, 3 * r:3 * r + 3],
                                     repq[:, r, :],
                                     start=(r == 0), stop=(r == 2),
                                     skip_group_check=True)
                lh = sb.tile([3, 512], F32R, tag="lh", bufs=2)
                nc.scalar.activation(lh[:], ps_lh[:], AF.Identity)
                Lq = []
                X4 = []
                for m in range(4):
                    ps_L = ps.tile([128, GCP], F32, tag="ps_m", bufs=2)
                    nc.tensor.matmul(ps_L[:], lh[:, 128 * m:128 * (m + 1)], iota3[:],
                                     start=True, stop=True)
                    Lqm = sb.tile([128, GCP], F32R, tag=f"Lq{m}", name=f"Lqm{m}", bufs=1)
                    nc.scalar.activation(Lqm[:], ps_L[:], AF.Identity)
                    Lq.append(Lqm)
                    ps_X = ps.tile([128, 512], F32, tag="ps_m", bufs=2)
                    nc.tensor.matmul(ps_X[:], repx[:, m, 0, :],
                                     ones16[:], start=True, stop=False, skip_group_check=True)
                    nc.tensor.matmul(ps_X[:], repx[:, m, 1, :], xh_s_g,
                                     start=False, stop=False, skip_group_check=True)
                    # repx r=2 entries are -0.5: folds the -u^2/2 scaling
                    nc.tensor.matmul(ps_X[:], repx[:, m, 2, :], xsq_g,
                                     start=False, stop=True, skip_group_check=True)
                    X4m = sb.tile([128, 512], F32R, tag=f"X4{m}", name=f"X4m{m}", bufs=1)
                    nc.scalar.activation(X4m[:], ps_X[:], AF.Identity)
                    X4.append(X4m)
                return Lq, X4

            # ---------------- one KDE slot ----------------
            def slot(u, k, Lq, X4):
                m, s = divmod(k, 4)
                sidx_ = u * DPC + k
                psu = ps.tile([GC, 512], F32, tag="psu", bufs=2)
                nc.tensor.matmul(psu[:], Lq[m][32 * s:32 * s + 3, 0:GC],
                                 X4[m][32 * s:32 * s + 3, :], start=True, stop=True,
                                 tile_position=(32 * s, 0), skip_group_check=True)
                dump = sb.tile([GC, 512], BF16, tag="dump", bufs=2)
                nc.scalar.activation(dump[:], psu[:], AF.Exp,
                                     accum_out=densC[:, sidx_:sidx_ + 1])

            # ---------------- GAT prologue ----------------
            def gat_prologue(l):
                cur = curT[l]
                for b in range(4):
                    pxh = ps.tile([128, 512], F32, tag="ps_m", bufs=2)
                    nc.tensor.matmul(pxh[:], W[l][:], cur[:, 512 * b:512 * (b + 1)],
                                     start=True, stop=True)
                    nc.vector.tensor_copy(xhT[:, 512 * b:512 * (b + 1)], pxh[:])
                for t in range(16):
                    pal = ps.tile([128, 8], F32, tag="ps_m", bufs=2)
                    nc.tensor.matmul(pal[:], xhT[:, 128 * t:128 * (t + 1)], Av[l][:],
                                     start=True, stop=True)
                    nc.vector.tensor_copy(al[:, t, :], pal[:])
                    pxr = ps.tile([128, 128], F32, tag="ps_m", bufs=2)
                    nc.tensor.transpose(pxr[:], xhT[:, 128 * t:128 * (t + 1)], id128[:])
                    nc.vector.tensor_copy(
                        stage[:, t, 0:132].rearrange("p (h c) -> p h c", h=4)[:, :, 0:32],
                        pxr[:].rearrange("p (h c) -> p h c", h=4))
                # al_s -> bf16 table cols 132:140 viewed as f32 x4
                nc.vector.tensor_copy(stage[:, :, 132:140].bitcast(F32), al[:, :, 0:4])
                nc.vector.tensor_copy(stage_d[:, :, 0:8].bitcast(F32), al[:, :, 4:8])
                nc.sync.dma_start(out=xh_al_hbm[:].rearrange("(t p) d -> p t d", p=128),
                                  in_=stage[:])
                nc.sync.dma_start(out=ald_hbm[:].rearrange("(t p) d -> p t d", p=128),
                                  in_=stage_d[:])

            # ---------------- GAT edge batches ----------------
            def finish_tile(l, t, raw):
                rawv = raw[:].rearrange("p (h c) -> p h c", h=4)
                rd = sb.tile([128, 4], F32, tag="rd")
                nc.vector.tensor_scalar(out=rd[:], in0=rawv[:, :, 32], scalar1=1e-16,
                                        scalar2=None, op0=ALU.add)
                nc.vector.reciprocal(out=rd[:], in_=rd[:])
                o = sb.tile([128, HC], F32, tag="otile", bufs=2)
                nc.vector.tensor_tensor(
                    out=o[:].rearrange("p (h c2) -> p h c2", h=4),
                    in0=rawv[:, :, 0:32],
                    in1=rd[:].rearrange("p h -> p h ()").to_broadcast([128, 4, 32]),
                    op=ALU.mult)
                pt = ps.tile([128, 128], F32, tag="ps_m", bufs=2)
                nc.tensor.transpose(pt[:], o[:], id128[:])
                nc.scalar.activation(curT[l + 1][:, 128 * t:128 * (t + 1)], pt[:],
                                     AF.Relu if l == 0 else AF.Identity,
                                     bias=bcol[l][:, 0:1])

            def gat_edges(l, chunk_tile):
                """Generator: yields after each emitted batch."""
                raw = None
                cur_t = -1
                for b0 in range(0, C, BATCH):
                    cn = min(BATCH, C - b0)
                    gwin = sb.tile([128, BATCH, TBLW], BF16, tag="gwin", bufs=2)
                    nc.gpsimd.dma_gather(gwin[:], xh_al_hbm[:],
                                         sidx[:, b0 * 8:(b0 + BATCH) * 8],
                                         BATCH * 128, BATCH * 128, TBLW, queue_num=0)
                    aldw = sb.tile([128, BATCH, ALDW], BF16, tag="aldw", bufs=2)
                    nc.gpsimd.dma_gather(aldw[:], ald_hbm[:],
                                         didx[:, b0 * 8:(b0 + BATCH) * 8],
                                         BATCH * 128, BATCH * 128, ALDW, queue_num=0)
                    # z = al_s[src] + al_d[dst]; leaky-relu; +maskneg; exp -> bf16
                    z = sb.tile([128, BATCH, 4], F32, tag="z", bufs=2)
                    nc.vector.tensor_tensor(out=z[:, 0:cn, :],
                                            in0=gwin[:, 0:cn, 132:140].bitcast(F32),
                                            in1=aldw[:, 0:cn, 0:8].bitcast(F32), op=ALU.add)
                    # leaky-relu fused: max(z, 0.2*z)
                    zl = sb.tile([128, BATCH, 4], F32, tag="zl", bufs=2)
                    nc.vector.scalar_tensor_tensor(
                        out=zl[:, 0:cn, :], in0=z[:, 0:cn, :], scalar=NEG_SLOPE,
                        in1=z[:, 0:cn, :], op0=ALU.mult, op1=ALU.max)
                    p_r = sb.tile([128, BATCH, 4], BF16, tag="p_r", bufs=2)
                    nc.scalar.activation(p_r[:, 0:cn, :], zl[:, 0:cn, :], AF.Exp)
                    nc.vector.tensor_tensor(
                        out=p_r[:, 0:cn, :], in0=p_r[:, 0:cn, :],
                        in1=mneg[:, b0:b0 + cn].rearrange("p c -> p c ()").to_broadcast([128, cn, 4]),
                        op=ALU.mult)
                    # one-hot [e, d] for the whole batch
                    OH = sb.tile([128, BATCH, 128], BF16, tag="OH", bufs=2)
                    nc.vector.tensor_tensor(
                        out=OH[:, 0:cn, :],
                        in0=dlfb[:, b0:b0 + cn].rearrange("p c -> p c ()").to_broadcast([128, cn, 128]),
                        in1=iotaF[:].rearrange("p f -> p () f").to_broadcast([128, cn, 128]),
                        op=ALU.is_equal)
                    # sxh = gathered (feat|1.0) * alpha  (4x33 interleave)
                    sxh = sb.tile([128, BATCH, 132], BF16, tag="sxh", bufs=2)
                    nc.vector.tensor_tensor(
                        out=sxh[:, 0:cn, :].rearrange("p b (h c) -> p b h c", h=4),
                        in0=gwin[:, 0:cn, 0:132].rearrange("p b (h c) -> p b h c", h=4),
                        in1=p_r[:, 0:cn, :].rearrange("p b h -> p b h ()").to_broadcast([128, cn, 4, 33]),
                        op=ALU.mult)
                    for ci in range(cn):
                        c = b0 + ci
                        t = int(chunk_tile[c])
                        first = (c == 0) or (int(chunk_tile[c - 1]) != t)
                        last = (c == C - 1) or (int(chunk_tile[c + 1]) != t)
                        if first:
                            if raw is not None:
                                finish_tile(l, cur_t, raw)
                            raw = ps.tile([128, 132], F32, tag="raw",
                                          padded_shape=[128, 512], bufs=2)
                            cur_t = t
                        nc.tensor.matmul(raw[:], OH[:, ci, :], sxh[:, ci, :],
                                         start=first, stop=last, skip_group_check=True)
                    yield
                if raw is not None:
                    finish_tile(l, cur_t, raw)

            # ---------------- quantile stage ----------------
            def quantiles():
                qs = np.linspace(0.0, 1.0, Q)
                tws = [min(128, GEFF - 128 * t) for t in range(4)]
                # cdf at fine grid, [fine part (4x128), slot free] via PE
                for t in range(4):
                    tw = tws[t]
                    cps = ps.tile([128, 512], F32, tag="ps_m", bufs=2)
                    nc.tensor.matmul(cps[0:tw, 0:SLOTS], triM[:, 128 * t:128 * t + tw],
                                     densC[:], start=True, stop=True)
                    nc.scalar.activation(cdfn[0:tw, t, :], cps[0:tw, 0:SLOTS], AF.Identity)
                # normalize by cdf[last]: extract via 1-col matmul to partition 0
                lastp = ps.tile([2, SLOTS], F32, tag="qrow", bufs=2)
                nc.tensor.matmul(lastp[0:1, :], triM[:, GEFF - 1:GEFF], densC[:],
                                 start=True, stop=True)
                rec = sb.tile([1, SLOTS], F32, tag="rec")
                nc.vector.reciprocal(out=rec[:], in_=lastp[0:1, :])
                r128 = ps.tile([128, 512], F32, tag="ps_m", bufs=2)
                nc.tensor.matmul(r128[:, 0:SLOTS], onesrow[:], rec[:],
                                 start=True, stop=True)
                for t in range(4):
                    tw = tws[t]
                    nc.vector.tensor_tensor(out=cdfn[0:tw, t, :], in0=cdfn[0:tw, t, :],
                                            in1=r128[0:tw, 0:SLOTS], op=ALU.mult)
                for qi in range(Q):
                    qrow = ps.tile([2, SLOTS], F32, tag="qrow", bufs=2)
                    for t in range(4):
                        tw = tws[t]
                        d1 = sb.tile([128, SLOTS], F32, tag="d1", bufs=2)
                        nc.vector.tensor_scalar(out=d1[0:tw, :], in0=cdfn[0:tw, t, :],
                                                scalar1=float(-qs[qi]), scalar2=None,
                                                op0=ALU.add)
                        nc.vector.tensor_scalar(out=d1[0:tw, :].bitcast(I32),
                                                in0=d1[0:tw, :].bitcast(I32),
                                                scalar1=0x7FFFFFFF, scalar2=None,
                                                op0=ALU.bitwise_and)
                        w = sb.tile([128, SLOTS], F32, tag="wt", bufs=2)
                        nc.scalar.activation(w[0:tw, :], d1[0:tw, :], AF.Sigmoid,
                                             scale=-100.0)
                        nc.tensor.matmul(qrow[:], oiT[0:tw, :, t], w[0:tw, :],
                                         start=(t == 0), stop=(t == 3),
                                         skip_group_check=True)
                    st2 = sb.tile([2, SLOTS], F32, tag="st2", bufs=2)
                    nc.scalar.activation(st2[:], qrow[:], AF.Identity)
                    nc.sync.dma_start(out=s0t1_out[2 * qi:2 * qi + 2, :], in_=st2[:])

            # ---------------- main schedule ----------------
            phases = os.environ.get("KPHASES", "all")
            if phases != "all":
                nc.gpsimd.memset(densC[:], 0.0)
                nc.gpsimd.memset(curT[1][:], 0.0)
                nc.gpsimd.memset(curT[2][:], 0.0)

            def layer(l, with_gat):
                stats_l, xsall = stats_phase(l)
                gen = None
                if with_gat:
                    gat_prologue(l)
                    gen = gat_edges(l, chunk_tile)
                nbatch_total = (C + BATCH - 1) // BATCH
                emitted = 0
                # interleave: 4 graphs x (prep + 16 slots); emit edge batches
                # between slots so GAT's Pool/DMA work overlaps readout compute
                points = 4 * (1 + 16)
                per_point = nbatch_total / points if with_gat else 0.0
                acc = 0.0

                def drain():
                    nonlocal emitted, acc
                    acc += per_point
                    while gen is not None and emitted < min(nbatch_total, int(round(acc))):
                        try:
                            next(gen); emitted += 1
                        except StopIteration:
                            return

                for g in range(G):
                    u = l * G + g
                    cstack_g, xh_s_g, xsq_g = derive_graph(l, g, xsall, *stats_l[g])
                    Lq, X4 = unit_prep(l, g, cstack_g, xh_s_g, xsq_g)
                    drain()
                    for k in range(DPC):
                        slot(u, k, Lq, X4)
                        drain()
                if with_gat:
                    for _ in gen:
                        pass

            for _ in range(reps):
                if phases == "r0":
                    layer(0, False)
                elif phases == "r0g0":
                    layer(0, True)
                else:
                    layer(0, True)
                    layer(1, True)
                    layer(2, False)
                quantiles()

            if KDBG:
                nc.sync.dma_start(out=dbg_dens[:], in_=densC[:])
                nc.sync.dma_start(out=dbg_cdfn[:], in_=cdfn[:].rearrange("p t s -> p (t s)"))
            nc.sync.dma_start(out=pmean_out[:], in_=pmean[:])
            nc.sync.dma_start(out=pmax_out[:], in_=pmax[:])
    nc.compile()
    return nc


_CACHE = {}


def _get_program(C, chunk_tile, reps=1):
    key = (C, tuple(chunk_tile.tolist()), reps,
           os.environ.get("KPHASES", "all"), os.environ.get("KDEBUG"))
    if key not in _CACHE:
        _CACHE[key] = build_program(C, chunk_tile, reps)
    return _CACHE[key]


def _host_inputs(inputs, s_idx, d_idx, dlf, maskneg, C):
    x = np.asarray(inputs["x"], np.float32)
    repq = np.zeros((DPC, 3, 512), np.float32)
    repx = np.zeros((DPC, 4, 3, 128), np.float32)
    for k in range(DPC):
        m, s = divmod(k, 4)
        for r in range(3):
            repq[k, r, 128 * m + 32 * s + r] = 1.0
            repx[k, m, r, 32 * s + r] = -0.5 if r == 2 else 1.0
    import ml_dtypes
    kk = np.arange(GCP, dtype=np.float64) - (GC - 1) / 2.0

    def wrap16(idx):
        # idx [128, C] int32 -> [128, CW] i16: global edge j=c*128+e at [j%16, j//16],
        # replicated across the 8 Q7 cores (partition blocks of 16)
        Cn = idx.shape[1]
        CW = ((Cn + BATCH - 1) // BATCH) * BATCH * 8
        flat = idx.T.ravel()                       # j = c*128+e order
        t = np.zeros((16, CW), np.int16)
        jj = np.arange(Cn * 128)
        t[jj % 16, jj // 16] = flat.astype(np.int16)
        return np.tile(t, (8, 1))

    # fused interp+cumsum matrix: cdf500 = densC^T @ triM
    M = np.zeros((GC, GRID))
    pos = np.arange(GRID) * (GC - 1) / (GRID - 1)
    lo = np.floor(pos).astype(int)
    wf = pos - lo
    hi = np.minimum(lo + 1, GC - 1)
    np.add.at(M, (lo, np.arange(GRID)), 1 - wf)
    np.add.at(M, (hi, np.arange(GRID)), wf)
    triM = np.cumsum(M, axis=1).astype(np.float32)

    oiT = np.zeros((128, 2, 4), np.float32)
    oiT[:, 0, :] = 1.0
    oiT[:, 1, :] = (np.arange(128, dtype=np.float32)[:, None]
                    + 128.0 * np.arange(4, dtype=np.float32)[None, :])

    qsv = np.linspace(0.0, 1.0, Q)
    qbias = np.zeros((128, 2 * Q + 1), np.float32)
    qbias[:, 0:2 * Q:2] = 100.0 * qsv[None, :]
    qbias[:, 1:2 * Q:2] = -100.0 * qsv[None, :]
    qbias[:, 2 * Q] = 1e-8

    im_base = dict(
        repq=repq, repx=repx,
        xT=np.ascontiguousarray(x.T),
        sidx=wrap16(s_idx), didx=wrap16(d_idx),
        dlfb=dlf.astype(ml_dtypes.bfloat16),
        mneg=maskneg,
        iota3=np.stack([np.ones(GCP), kk, kk ** 2]).astype(np.float32),
        triM=triM,
        oiT=oiT,
        qbias=qbias,
        iotaF=np.tile(np.arange(128, dtype=np.float32)[None, :], (128, 1)).astype(ml_dtypes.bfloat16),
    )
    for l in range(2):
        A = np.zeros((128, 8), np.float32)
        as_l = np.asarray(inputs[f"as{l}"], np.float32)
        ad_l = np.asarray(inputs[f"ad{l}"], np.float32)
        for h in range(HEADS):
            A[h * HID:(h + 1) * HID, h] = as_l[h]
            A[h * HID:(h + 1) * HID, 4 + h] = ad_l[h]
        im_base[f"W{l}"] = np.asarray(inputs[f"W{l}"], np.float32)
        im_base[f"Av{l}"] = A
        im_base[f"bcol{l}"] = np.asarray(inputs[f"b{l}"], np.float32).reshape(128, 1)
    in_maps = []
    for c in range(N_CORES):
        selm = np.zeros((128, DPC), np.float32)
        for k in range(DPC):
            selm[DPC * c + k, k] = 1.0
        in_maps.append({**im_base, "sel": selm})
    return in_maps


def kernel(**inputs) -> np.ndarray:
    from concourse.bass_utils import run_bass_kernel_spmd
    s_idx, d_idx, dlf, maskneg, chunk_tile = _edge_prep(np.asarray(inputs["edge_index"]))
    C = s_idx.shape[1]
    nc = _get_program(C, chunk_tile)
    in_maps = _host_inputs(inputs, s_idx, d_idx, dlf, maskneg, C)
    res = run_bass_kernel_spmd(nc, in_maps, list(range(N_CORES))).results
    return _assemble(inputs, res)


def _assemble(inputs, res):
    # kf[l, g, d, q]
    kf = np.zeros((3, G, 128, Q), np.float64)
    for c in range(N_CORES):
        s0t1 = np.asarray(res[c]["s0t1"], np.float64)    # [2Q, 192]
        S0, T1 = s0t1[0::2, :], s0t1[1::2, :]            # [Q, 192]
        mndl2 = np.asarray(res[c]["mndl2"], np.float64)  # [16, 24]
        mn = mndl2[:, 0::2].T.reshape(-1)                # [192] slot-ordered
        dl = mndl2[:, 1::2].T.reshape(-1)
        qvT = (mn[None, :] * S0 + dl[None, :] * T1) / (S0 + 1e-8)
        for u in range(UNITS):
            l, g = divmod(u, G)
            kf[l, g, DPC * c:DPC * (c + 1), :] = qvT[:, u * DPC:(u + 1) * DPC].T
    pmean = res[0]["pmean"] / 512.0          # [128, 12]
    pmax = res[0]["pmax"]
    pool_w = np.asarray(inputs["pool_w"], np.float64)
    beta = np.asarray(inputs["beta"], np.float64)
    h0 = float(np.asarray(inputs["h0"]).reshape(-1)[0])
    h_list, k_list = [], []
    for l in range(3):
        wp = (pool_w[0] * pmean[:, l * G:(l + 1) * G] + pool_w[1] * pmax[:, l * G:(l + 1) * G]).T
        lpW = np.asarray(inputs[f"lpW{l}"], np.float64)
        lpb = np.asarray(inputs[f"lpb{l}"], np.float64)
        h_list.append(wp @ lpW + lpb)
        kW = np.asarray(inputs[f"kW{l}"], np.float64)
        kb = np.asarray(inputs[f"kb{l}"], np.float64)
        k_list.append(kf[l].reshape(G, 128 * Q) @ kW + kb)
    main_out = np.mean(h_list, axis=0)
    kde_out = np.mean(k_list, axis=0)
    risk = (main_out + kde_out) @ beta + h0
    return risk.astype(np.float32)


# revision 49
# speedup vs baseline: 1.0817x; 1.0817x over previous
"""Trainium2 Bass kernel for nn_GAT_KDE_14766097563859.

2-layer GAT over a 2048-node graph + per-(graph,layer) KDE soft-quantile
readouts. SPMD over 8 NeuronCores: GAT replicated, KDE sharded by feature dim
(each core owns 16 of 128 dims for all 12 (layer,graph) units = 192 slots).

v3 (over the v2 baseline):
- density evaluated on a 125-point coarse grid; cumsum+linear-interp back to
  the 500-point cdf is folded into one constant [125,500] matmul matrix
  (linear maps compose), cutting exp/matmul volume 4x
- per-slot node-sum fused into the exp activation via accum_out (zero DVE)
- quantile stage flipped to [fine-grid part, slot free] orientation:
  sigma(-100|d|) = min(sigma(-100d), sigma(100d)) with the q-shift folded
  into the sigmoid bias; S0/T1 reductions over the grid run on PE
- PSUM->SBUF copies moved to Act; xh_s/xsq on Act (scale-AP / Square, with
  the -0.5 folded into the repx constant); per-graph xs kept, not re-derived
"""
import os
import sys
sys.path.insert(0, "/opt/trn_rl_repo")
import numpy as np

import concourse.bass as bass
import concourse.bacc as bacc
import concourse.mybir as mybir
from concourse.tile import TileContext
from concourse.masks import make_identity
from concourse import library_config

F32 = mybir.dt.float32
F32R = mybir.dt.float32r
BF16 = mybir.dt.bfloat16
I32 = mybir.dt.int32
I16 = mybir.dt.int16
AF = mybir.ActivationFunctionType
ALU = mybir.AluOpType
AX = mybir.AxisListType

G, NG, N, E = 4, 512, 2048, 32768
IN_DIM, HID, HEADS, HC, OUT_DIM = 128, 32, 4, 128, 32
N_LAYERS, GRID, Q, NEG_SLOPE = 2, 500, 20, 0.2
N_CORES = 8
UNITS = (N_LAYERS + 1) * G            # 12, unit u = l*G + g
DPC = 16                              # dims per core
SLOTS = UNITS * DPC                   # 192
GEFF = 500                            # fine grid (matches reference GRID)
GC = 125                              # coarse density grid
GCP = 128                             # GC padded (fp32r matmul needs free%4==0)
BATCH = 8                             # chunks per gather batch (1024 idxs = SWDGE ring cap)
TBLW = 256                            # bf16 cols of xh_al row (512B, dma_gather needs %256B)
ALDW = 128                            # bf16 cols of al_d row (256B)


def _edge_prep(edge_index):
    src = edge_index[0].astype(np.int64)
    dst = edge_index[1].astype(np.int64)
    s_all = np.concatenate([src, np.arange(N)])
    d_all = np.concatenate([dst, np.arange(N)])
    order = np.argsort(d_all, kind="stable")
    s_s, d_s = s_all[order], d_all[order]
    cs, cd, ct, cm = [], [], [], []
    for t in range(16):
        sel = (d_s // 128) == t
        se, de = s_s[sel], d_s[sel]
        ne = len(se)
        npad = (-ne) % 128
        se = np.concatenate([se, np.zeros(npad, np.int64)])
        de = np.concatenate([de, np.full(npad, t * 128, np.int64)])
        rm = np.concatenate([np.ones(ne, bool), np.zeros(npad, bool)])
        for c0 in range(0, len(se), 128):
            cs.append(se[c0:c0 + 128]); cd.append(de[c0:c0 + 128])
            cm.append(rm[c0:c0 + 128]); ct.append(t)
    C = len(cs)
    s_idx = np.stack(cs, 1).astype(np.int32)                  # [128, C] src ids
    d_idx = np.stack(cd, 1).astype(np.int32)                  # [128, C] dst ids
    dlf = (np.stack(cd, 1) - np.asarray(ct)[None, :] * 128).astype(np.float32)
    maskneg = np.stack(cm, 1).astype(np.float32)  # 1 valid, 0 pad
    return s_idx, d_idx, dlf, maskneg, np.asarray(ct, np.int32)


def build_program(C, chunk_tile, reps=1):
    nc = bacc.Bacc(None, target_bir_lowering=False, debug=True)

    xT_in = nc.declare_dram_parameter("xT", [128, N], F32, isOutput=False)
    Wi = [nc.declare_dram_parameter(f"W{l}", [128, HC], F32, isOutput=False) for l in range(2)]
    Avi = [nc.declare_dram_parameter(f"Av{l}", [128, 8], F32, isOutput=False) for l in range(2)]
    bci = [nc.declare_dram_parameter(f"bcol{l}", [128, 1], F32, isOutput=False) for l in range(2)]
    sel_in = nc.declare_dram_parameter("sel", [128, DPC], F32, isOutput=False)
    iota3_in = nc.declare_dram_parameter("iota3", [3, GCP], F32R, isOutput=False)
    triM_in = nc.declare_dram_parameter("triM", [GC, 512], F32, isOutput=False)
    oiT_in = nc.declare_dram_parameter("oiT", [128, 2, 4], F32, isOutput=False)
    qbias_in = nc.declare_dram_parameter("qbias", [128, 2 * Q + 1], F32, isOutput=False)
    iotaF_in = nc.declare_dram_parameter("iotaF", [128, 128], BF16, isOutput=False)
    CW = ((C + BATCH - 1) // BATCH) * BATCH * 8
    sidx_in = nc.declare_dram_parameter("sidx", [128, CW], I16, isOutput=False)
    didx_in = nc.declare_dram_parameter("didx", [128, CW], I16, isOutput=False)
    dlfb_in = nc.declare_dram_parameter("dlfb", [128, C], BF16, isOutput=False)
    mneg_in = nc.declare_dram_parameter("mneg", [128, C], F32, isOutput=False)
    repq_in = nc.declare_dram_parameter("repq", [DPC, 3, 512], F32R, isOutput=False)
    repx_in = nc.declare_dram_parameter("repx", [DPC, 4, 3, 128], F32R, isOutput=False)

    s0t1_out = nc.declare_dram_parameter("s0t1", [2 * Q, SLOTS], F32, isOutput=True)
    mndl_out = nc.declare_dram_parameter("mndl2", [DPC, 2 * UNITS], F32, isOutput=True)
    pmean_out = nc.declare_dram_parameter("pmean", [128, UNITS], F32, isOutput=True)
    pmax_out = nc.declare_dram_parameter("pmax", [128, UNITS], F32, isOutput=True)
    KDBG = os.environ.get("KDEBUG") == "1"
    if KDBG:
        dbg_dens = nc.declare_dram_parameter("dbg_dens", [GC, SLOTS], F32, isOutput=True)

    xh_al_hbm = nc.dram_tensor("xh_al", [N, TBLW], BF16)
    ald_hbm = nc.dram_tensor("ald", [N, ALDW], BF16)

    with TileContext(nc) as tc:
        with (
            tc.tile_pool(name="cst", bufs=1) as cst,
            tc.tile_pool(name="wrk", bufs=2) as sb,
            tc.tile_pool(name="ps", bufs=1, space="PSUM") as ps,
        ):
            # ---------------- resident state ----------------
            curT = [cst.tile([128, N], F32, tag=f"curT{l}", name=f"curT{l}") for l in range(3)]
            W = [cst.tile([128, HC], F32, tag=f"Wt{l}", name=f"Wt{l}") for l in range(2)]
            Av = [cst.tile([128, 8], F32, tag=f"Avt{l}", name=f"Avt{l}") for l in range(2)]
            bcol = [cst.tile([128, 1], F32, tag=f"bct{l}", name=f"bct{l}") for l in range(2)]
            sel = cst.tile([128, DPC], F32)
            iota3 = cst.tile([3, GCP], F32R)
            triM = cst.tile([GC, 512], F32)
            oiT = cst.tile([128, 2, 4], F32)
            qbias = cst.tile([128, 2 * Q + 1], F32)
            iotaF = cst.tile([128, 128], BF16)
            sidx = cst.tile([128, CW], I16)
            didx = cst.tile([128, CW], I16)
            dlfb = cst.tile([128, C], BF16)
            mneg = cst.tile([128, C], F32)
            repq = cst.tile([DPC, 3, 512], F32R)
            repx = cst.tile([DPC, 4, 3, 128], F32R)
            ones16 = cst.tile([DPC, 512], F32R)
            onesrow = cst.tile([1, 128], F32)
            densC = cst.tile([GC, SLOTS], F32)
            pmean = cst.tile([128, UNITS], F32)
            pmax = cst.tile([128, UNITS], F32)
            id128 = cst.tile([128, 128], F32)
            xhT = cst.tile([128, N], F32)
            al = cst.tile([128, 16, 8], F32)
            stage = cst.tile([128, 16, TBLW], BF16)
            stage_d = cst.tile([128, 16, ALDW], BF16)
            nc.gpsimd.load_library(library_config.mlp)
            make_identity(nc, id128[:])
            nc.gpsimd.dma_start(out=curT[0][:], in_=xT_in[:])
            for l in range(2):
                nc.gpsimd.dma_start(out=W[l][:], in_=Wi[l][:])
                nc.gpsimd.dma_start(out=Av[l][:], in_=Avi[l][:])
                nc.gpsimd.dma_start(out=bcol[l][:], in_=bci[l][:])
            nc.gpsimd.dma_start(out=sel[:], in_=sel_in[:])
            nc.gpsimd.dma_start(out=iota3[:], in_=iota3_in[:])
            nc.gpsimd.dma_start(out=triM[:], in_=triM_in[:])
            nc.gpsimd.dma_start(out=oiT[:], in_=oiT_in[:])
            nc.gpsimd.dma_start(out=qbias[:], in_=qbias_in[:])
            nc.gpsimd.dma_start(out=iotaF[:], in_=iotaF_in[:])
            nc.gpsimd.dma_start(out=sidx[:], in_=sidx_in[:])
            nc.gpsimd.dma_start(out=didx[:], in_=didx_in[:])
            nc.gpsimd.dma_start(out=dlfb[:], in_=dlfb_in[:])
            nc.gpsimd.dma_start(out=mneg[:], in_=mneg_in[:])
            nc.gpsimd.dma_start(out=repq[:], in_=repq_in[:])
            nc.gpsimd.dma_start(out=repx[:], in_=repx_in[:])
            nc.gpsimd.memset(ones16[:].bitcast(F32), 1.0)
            nc.gpsimd.memset(onesrow[:], 1.0)
            nc.gpsimd.memset(stage[:].bitcast(mybir.dt.uint16), 0)
            nc.gpsimd.memset(stage_d[:].bitcast(mybir.dt.uint16), 0)
            # constant 1.0 columns of the gather table (col 33h+32), set once
            nc.gpsimd.memset(stage[:, :, 0:132].rearrange("p t (h c) -> p t h c", c=33)[:, :, :, 32:33].bitcast(mybir.dt.uint16), 0x3F80)

            # ---------------- per-layer stats (all 4 graphs) ----------------
            def stats_phase(l):
                cur = curT[l]
                xsall = sb.tile([DPC, G, 512], F32, tag="xsall", bufs=2)
                per_g = []
                for g in range(G):
                    u = l * G + g
                    blk = cur[:, 512 * g:512 * (g + 1)]
                    nc.vector.tensor_reduce(out=pmean[:, u:u + 1], in_=blk, axis=AX.X, op=ALU.add)
                    nc.vector.tensor_reduce(out=pmax[:, u:u + 1], in_=blk, axis=AX.X, op=ALU.max)
                    ps_xs = ps.tile([DPC, 512], F32, tag="ps_m", bufs=2)
                    nc.tensor.matmul(ps_xs[:], sel[:], blk, start=True, stop=True)
                    xs = xsall[:, g, :]
                    nc.scalar.activation(xs, ps_xs[:], AF.Identity)
                    mn = sb.tile([DPC, 1], F32, tag=f"mn{g}", name=f"mn{g}", bufs=2)
                    mx = sb.tile([DPC, 1], F32, tag="mx")
                    sm = sb.tile([DPC, 1], F32, tag="sm")
                    sq = sb.tile([DPC, 1], F32, tag="sq")
                    nc.vector.tensor_reduce(out=mn[:], in_=xs, axis=AX.X, op=ALU.min)
                    nc.vector.tensor_reduce(out=mx[:], in_=xs, axis=AX.X, op=ALU.max)
                    nc.vector.tensor_reduce(out=sm[:], in_=xs, axis=AX.X, op=ALU.add)
                    xsq0 = sb.tile([DPC, 512], F32, tag="xsq0")
                    nc.vector.tensor_tensor(out=xsq0[:], in0=xs, in1=xs, op=ALU.mult)
                    nc.vector.tensor_reduce(out=sq[:], in_=xsq0[:], axis=AX.X, op=ALU.add)
                    var = sb.tile([DPC, 1], F32, tag=f"var{g}", name=f"var{g}", bufs=2)
                    mean = sb.tile([DPC, 1], F32, tag="mean")
                    nc.vector.tensor_scalar(out=mean[:], in0=sm[:], scalar1=1.0 / 512,
                                            scalar2=None, op0=ALU.mult)
                    nc.vector.tensor_scalar(out=var[:], in0=sq[:], scalar1=1.0 / 512,
                                            scalar2=None, op0=ALU.mult)
                    m2 = sb.tile([DPC, 1], F32, tag="m2")
                    nc.vector.tensor_tensor(out=m2[:], in0=mean[:], in1=mean[:], op=ALU.mult)
                    nc.vector.tensor_tensor(out=var[:], in0=var[:], in1=m2[:], op=ALU.subtract)
                    nc.vector.tensor_scalar(out=mn[:], in0=mn[:], scalar1=-1e-6, scalar2=None, op0=ALU.add)
                    nc.vector.tensor_scalar(out=mx[:], in0=mx[:], scalar1=1e-6, scalar2=None, op0=ALU.add)
                    dl = sb.tile([DPC, 1], F32, tag=f"dl{g}", name=f"dl{g}", bufs=2)
                    nc.vector.tensor_tensor(out=dl[:], in0=mx[:], in1=mn[:], op=ALU.subtract)
                    nc.vector.tensor_scalar(out=dl[:], in0=dl[:], scalar1=1.0 / (GEFF - 1), scalar2=None, op0=ALU.mult)
                    per_g.append((mn, dl, var))
                # batched sqrt on Act (one table region), mn/dl -> row layout
                out = []
                for g, (mn, dl, var) in enumerate(per_g):
                    u = l * G + g
                    std = sb.tile([DPC, 1], F32, tag=f"std{g}", name=f"std{g}", bufs=2)
                    nc.scalar.activation(std[:], var[:], AF.Sqrt)
                    nc.sync.dma_start(out=mndl_out[:, 2 * u:2 * u + 1], in_=mn[:])
                    nc.sync.dma_start(out=mndl_out[:, 2 * u + 1:2 * u + 2], in_=dl[:])
                    out.append((mn, dl, std))
                return out, xsall

            def derive_graph(l, g, xsall, mn, dl, std):
                """Per-graph slot-pass prep: build cstack + X rows from kept xs."""
                xs = xsall[:, g, :]
                h = sb.tile([DPC, 1], F32, tag="h")
                nc.vector.tensor_scalar(out=h[:], in0=std[:], scalar1=float(1e-8 / 3),
                                        scalar2=float(1.06 * 512 ** -0.2),
                                        op0=ALU.add, op1=ALU.mult)
                rh = sb.tile([DPC, 1], F32, tag="rh", bufs=2)
                nc.vector.reciprocal(out=rh[:], in_=h[:])
                # recentered basis: k' = k-(GC-1)/2, x' = (x-c)/h with c the
                # grid midpoint -> a0 = 0 and all poly terms stay O(span/2h),
                # minimizing f32r (tf32) rounding of the stored poly values
                a0 = sb.tile([DPC, 1], F32, tag="a0")
                a1 = sb.tile([DPC, 1], F32, tag="a1")
                nc.gpsimd.memset(a0[:], 0.0)
                # coarse grid step is (GEFF-1)/(GC-1) fine steps
                nc.vector.scalar_tensor_tensor(
                    out=a1[:], in0=dl[:], scalar=float((GEFF - 1) / (GC - 1)),
                    in1=rh[:], op0=ALU.mult, op1=ALU.mult)
                mnrh = sb.tile([DPC, 1], F32, tag="mnrh")
                nc.vector.tensor_tensor(out=mnrh[:], in0=mn[:], in1=rh[:], op=ALU.mult)
                biasc = sb.tile([DPC, 1], F32, tag="biasc", bufs=2)
                nc.vector.scalar_tensor_tensor(
                    out=biasc[:], in0=a1[:], scalar=-float((GC - 1) / 2),
                    in1=mnrh[:], op0=ALU.mult, op1=ALU.subtract)
                # cstack [16, 9]: r0=(q0,q1,q2) r1=(a0,a1,0) r2=(1,0,0)
                cstack = sb.tile([DPC, 9], F32R, tag="cs", bufs=2)
                nc.gpsimd.memset(cstack[:].bitcast(F32), 0.0)
                nc.vector.tensor_tensor(out=cstack[:, 0:1], in0=a0[:], in1=a0[:], op=ALU.mult)
                nc.vector.tensor_scalar(out=cstack[:, 0:1], in0=cstack[:, 0:1], scalar1=-0.5, scalar2=None, op0=ALU.mult)
                nc.vector.tensor_tensor(out=cstack[:, 1:2], in0=a0[:], in1=a1[:], op=ALU.mult)
                nc.vector.tensor_scalar(out=cstack[:, 1:2], in0=cstack[:, 1:2], scalar1=-1.0, scalar2=None, op0=ALU.mult)
                nc.vector.tensor_tensor(out=cstack[:, 2:3], in0=a1[:], in1=a1[:], op=ALU.mult)
                nc.vector.tensor_scalar(out=cstack[:, 2:3], in0=cstack[:, 2:3], scalar1=-0.5, scalar2=None, op0=ALU.mult)
                nc.vector.tensor_copy(cstack[:, 3:4], a0[:])
                nc.vector.tensor_copy(cstack[:, 4:5], a1[:])
                nc.gpsimd.memset(cstack[:, 6:7].bitcast(F32), 1.0)
                xh_s = sb.tile([DPC, 512], F32R, tag="xh_s", bufs=2)
                nc.scalar.activation(xh_s[:], xs, AF.Identity, scale=rh[:, 0:1],
                                     bias=biasc[:, 0:1])
                xsq = sb.tile([DPC, 512], F32R, tag="xsq", bufs=2)
                nc.scalar.activation(xsq[:], xh_s[:], AF.Square)
                return cstack, xh_s, xsq

            # ---------------- per-(unit) prep: Lq / X4 ----------------
            def unit_prep(l, g, cstack_g, xh_s_g, xsq_g):
                # lh [3, 512]: col 128m+32s+r' = cstack triple r' of slot k=4m+s
                ps_lh = ps.tile([3, 512], F32, tag="ps_m", bufs=2)
                for r in range(3):
                    nc.tensor.matmul(ps_lh[:], cstack_g[:, 3 * r:3 * r + 3],
                                     repq[:, r, :],
                                     start=(r == 0), stop=(r == 2),
                                     skip_group_check=True)
                lh = sb.tile([3, 512], F32R, tag="lh", bufs=2)
                nc.scalar.activation(lh[:], ps_lh[:], AF.Identity)
                Lq = []
                X4 = []
                for m in range(4):
                    ps_L = ps.tile([128, GCP], F32, tag="ps_m", bufs=2)
                    nc.tensor.matmul(ps_L[:], lh[:, 128 * m:128 * (m + 1)], iota3[:],
                                     start=True, stop=True)
                    Lqm = sb.tile([128, GCP], F32R, tag=f"Lq{m}", name=f"Lqm{m}", bufs=4)
                    nc.scalar.activation(Lqm[:], ps_L[:], AF.Identity)
                    Lq.append(Lqm)
                    ps_X = ps.tile([128, 512], F32, tag="ps_m", bufs=2)
                    nc.tensor.matmul(ps_X[:], repx[:, m, 0, :],
                                     ones16[:], start=True, stop=False, skip_group_check=True)
                    nc.tensor.matmul(ps_X[:], repx[:, m, 1, :], xh_s_g,
                                     start=False, stop=False, skip_group_check=True)
                    # repx r=2 entries are -0.5: folds the -u^2/2 scaling
                    nc.tensor.matmul(ps_X[:], repx[:, m, 2, :], xsq_g,
                                     start=False, stop=True, skip_group_check=True)
                    X4m = sb.tile([128, 512], F32R, tag=f"X4{m}", name=f"X4m{m}", bufs=4)
                    nc.scalar.activation(X4m[:], ps_X[:], AF.Identity)
                    X4.append(X4m)
                return Lq, X4

            # ---------------- one KDE slot ----------------
            def slot(u, k, Lq, X4):
                m, s = divmod(k, 4)
                sidx_ = u * DPC + k
                psu = ps.tile([GC, 512], F32, tag="psu", bufs=2)
                nc.tensor.matmul(psu[:], Lq[m][32 * s:32 * s + 3, 0:GC],
                                 X4[m][32 * s:32 * s + 3, :], start=True, stop=True,
                                 tile_position=(32 * s, 0), skip_group_check=True)
                dump = sb.tile([GC, 512], BF16, tag="dump", bufs=2)
                nc.scalar.activation(dump[:], psu[:], AF.Exp,
                                     accum_out=densC[:, sidx_:sidx_ + 1])

            # ---------------- GAT prologue ----------------
            def gat_prologue(l):
                cur = curT[l]
                for b in range(4):
                    pxh = ps.tile([128, 512], F32, tag="ps_m", bufs=2)
                    nc.tensor.matmul(pxh[:], W[l][:], cur[:, 512 * b:512 * (b + 1)],
                                     start=True, stop=True)
                    nc.vector.tensor_copy(xhT[:, 512 * b:512 * (b + 1)], pxh[:])
                for t in range(16):
                    pal = ps.tile([128, 8], F32, tag="ps_m", bufs=2)
                    nc.tensor.matmul(pal[:], xhT[:, 128 * t:128 * (t + 1)], Av[l][:],
                                     start=True, stop=True)
                    nc.vector.tensor_copy(al[:, t, :], pal[:])
                    pxr = ps.tile([128, 128], F32, tag="ps_m", bufs=2)
                    nc.tensor.transpose(pxr[:], xhT[:, 128 * t:128 * (t + 1)], id128[:])
                    nc.vector.tensor_copy(
                        stage[:, t, 0:132].rearrange("p (h c) -> p h c", h=4)[:, :, 0:32],
                        pxr[:].rearrange("p (h c) -> p h c", h=4))
                # al_s -> bf16 table cols 132:140 viewed as f32 x4
                nc.vector.tensor_copy(stage[:, :, 132:140].bitcast(F32), al[:, :, 0:4])
                nc.vector.tensor_copy(stage_d[:, :, 0:8].bitcast(F32), al[:, :, 4:8])
                nc.sync.dma_start(out=xh_al_hbm[:].rearrange("(t p) d -> p t d", p=128),
                                  in_=stage[:])
                nc.sync.dma_start(out=ald_hbm[:].rearrange("(t p) d -> p t d", p=128),
                                  in_=stage_d[:])

            # ---------------- GAT edge batches ----------------
            def finish_tile(l, t, raw):
                rawv = raw[:].rearrange("p (h c) -> p h c", h=4)
                rd = sb.tile([128, 4], F32, tag="rd")
                nc.vector.tensor_scalar(out=rd[:], in0=rawv[:, :, 32], scalar1=1e-16,
                                        scalar2=None, op0=ALU.add)
                nc.vector.reciprocal(out=rd[:], in_=rd[:])
                o = sb.tile([128, HC], F32, tag="otile", bufs=2)
                nc.vector.tensor_tensor(
                    out=o[:].rearrange("p (h c2) -> p h c2", h=4),
                    in0=rawv[:, :, 0:32],
                    in1=rd[:].rearrange("p h -> p h ()").to_broadcast([128, 4, 32]),
                    op=ALU.mult)
                pt = ps.tile([128, 128], F32, tag="ps_m", bufs=2)
                nc.tensor.transpose(pt[:], o[:], id128[:])
                nc.scalar.activation(curT[l + 1][:, 128 * t:128 * (t + 1)], pt[:],
                                     AF.Relu if l == 0 else AF.Identity,
                                     bias=bcol[l][:, 0:1])

            def gat_edges(l, chunk_tile):
                """Generator: yields after each emitted batch."""
                raw = None
                cur_t = -1
                for b0 in range(0, C, BATCH):
                    cn = min(BATCH, C - b0)
                    gwin = sb.tile([128, BATCH, TBLW], BF16, tag="gwin", bufs=2)
                    nc.gpsimd.dma_gather(gwin[:], xh_al_hbm[:],
                                         sidx[:, b0 * 8:(b0 + BATCH) * 8],
                                         BATCH * 128, BATCH * 128, TBLW, queue_num=0)
                    aldw = sb.tile([128, BATCH, ALDW], BF16, tag="aldw", bufs=2)
                    nc.gpsimd.dma_gather(aldw[:], ald_hbm[:],
                                         didx[:, b0 * 8:(b0 + BATCH) * 8],
                                         BATCH * 128, BATCH * 128, ALDW, queue_num=0)
                    # z = al_s[src] + al_d[dst]; leaky-relu; +maskneg; exp -> bf16
                    z = sb.tile([128, BATCH, 4], F32, tag="z", bufs=2)
                    nc.vector.tensor_tensor(out=z[:, 0:cn, :],
                                            in0=gwin[:, 0:cn, 132:140].bitcast(F32),
                                            in1=aldw[:, 0:cn, 0:8].bitcast(F32), op=ALU.add)
                    # leaky-relu fused: max(z, 0.2*z)
                    zl = sb.tile([128, BATCH, 4], F32, tag="zl", bufs=2)
                    nc.vector.scalar_tensor_tensor(
                        out=zl[:, 0:cn, :], in0=z[:, 0:cn, :], scalar=NEG_SLOPE,
                        in1=z[:, 0:cn, :], op0=ALU.mult, op1=ALU.max)
                    p_r = sb.tile([128, BATCH, 4], BF16, tag="p_r", bufs=2)
                    nc.scalar.activation(p_r[:, 0:cn, :], zl[:, 0:cn, :], AF.Exp)
                    nc.vector.tensor_tensor(
                        out=p_r[:, 0:cn, :], in0=p_r[:, 0:cn, :],
                        in1=mneg[:, b0:b0 + cn].rearrange("p c -> p c ()").to_broadcast([128, cn, 4]),
                        op=ALU.mult)
                    # one-hot [e, d] for the whole batch
                    OH = sb.tile([128, BATCH, 128], BF16, tag="OH", bufs=2)
                    nc.vector.tensor_tensor(
                        out=OH[:, 0:cn, :],
                        in0=dlfb[:, b0:b0 + cn].rearrange("p c -> p c ()").to_broadcast([128, cn, 128]),
                        in1=iotaF[:].rearrange("p f -> p () f").to_broadcast([128, cn, 128]),
                        op=ALU.is_equal)
                    # sxh = gathered (feat|1.0) * alpha  (4x33 interleave)
                    sxh = sb.tile([128, BATCH, 132], BF16, tag="sxh", bufs=2)
                    nc.vector.tensor_tensor(
                        out=sxh[:, 0:cn, :].rearrange("p b (h c) -> p b h c", h=4),
                        in0=gwin[:, 0:cn, 0:132].rearrange("p b (h c) -> p b h c", h=4),
                        in1=p_r[:, 0:cn, :].rearrange("p b h -> p b h ()").to_broadcast([128, cn, 4, 33]),
                        op=ALU.mult)
                    for ci in range(cn):
                        c = b0 + ci
                        t = int(chunk_tile[c])
                        first = (c == 0) or (int(chunk_tile[c - 1]) != t)
                        last = (c == C - 1) or (int(chunk_tile[c + 1]) != t)
                        if first:
                            if raw is not None:
                                finish_tile(l, cur_t, raw)
                            raw = ps.tile([128, 132], F32, tag="raw",
                                          padded_shape=[128, 512], bufs=2)
                            cur_t = t
                        nc.tensor.matmul(raw[:], OH[:, ci, :], sxh[:, ci, :],
                                         start=first, stop=last, skip_group_check=True)
                    yield
                if raw is not None:
                    finish_tile(l, cur_t, raw)

            # ---------------- quantile stage (per-layer slice pass) ----------------
            def quantiles(s0, s1):
                qs = np.linspace(0.0, 1.0, Q)
                tws = [min(128, GEFF - 128 * t) for t in range(4)]
                SW = s1 - s0
                dC = densC[:, s0:s1]
                # cdf at fine grid, [fine part (4x128), slot-slice free] via PE
                cdfp = sb.tile([128, 4, SW], F32, tag="cdfp", bufs=2)
                for t in range(4):
                    cps = ps.tile([128, 512], F32, tag="ps_m", bufs=2)
                    nc.tensor.matmul(cps[:, 0:SW], triM[:, 128 * t:128 * (t + 1)],
                                     dC, start=True, stop=True)
                    nc.scalar.activation(cdfp[:, t, :], cps[:, 0:SW], AF.Identity)
                # normalize by cdf[last]: extract via 1-col matmul to partition 0
                lastp = ps.tile([2, SW], F32, tag="qrow", bufs=2)
                nc.tensor.matmul(lastp[0:1, :], triM[:, GEFF - 1:GEFF], dC,
                                 start=True, stop=True)
                rec = sb.tile([1, SW], F32, tag="rec", bufs=2)
                nc.vector.reciprocal(out=rec[:], in_=lastp[0:1, :])
                r128 = ps.tile([128, 512], F32, tag="ps_m", bufs=2)
                nc.tensor.matmul(r128[:, 0:SW], onesrow[:], rec[:],
                                 start=True, stop=True)
                for t in range(4):
                    nc.vector.tensor_tensor(out=cdfp[:, t, :], in0=cdfp[:, t, :],
                                            in1=r128[:, 0:SW], op=ALU.mult)
                for qi in range(Q):
                    qrow = ps.tile([2, SW], F32, tag="qrow", bufs=2)
                    # fused across the 4 grid tiles: [128, 4*SW] (t=3 rows
                    # 116:128 hold garbage; the per-t matmuls skip them)
                    d1 = sb.tile([128, 4, SW], F32, tag="d1", bufs=2)
                    nc.vector.tensor_scalar(out=d1[:].rearrange("p t s -> p (t s)"),
                                            in0=cdfp[:].rearrange("p t s -> p (t s)"),
                                            scalar1=float(-qs[qi]), scalar2=None,
                                            op0=ALU.add)
                    nc.vector.tensor_scalar(out=d1[:].rearrange("p t s -> p (t s)").bitcast(I32),
                                            in0=d1[:].rearrange("p t s -> p (t s)").bitcast(I32),
                                            scalar1=0x7FFFFFFF, scalar2=None,
                                            op0=ALU.bitwise_and)
                    w = sb.tile([128, 4, SW], F32, tag="wt", bufs=2)
                    nc.scalar.activation(w[:].rearrange("p t s -> p (t s)"),
                                         d1[:].rearrange("p t s -> p (t s)"),
                                         AF.Sigmoid, scale=-100.0)
                    for t in range(4):
                        tw = tws[t]
                        nc.tensor.matmul(qrow[:], oiT[0:tw, :, t], w[0:tw, t, :],
                                         start=(t == 0), stop=(t == 3),
                                         skip_group_check=True)
                    st2 = sb.tile([2, SW], F32, tag="st2", bufs=2)
                    nc.scalar.activation(st2[:], qrow[:], AF.Identity)
                    nc.sync.dma_start(out=s0t1_out[2 * qi:2 * qi + 2, s0:s1], in_=st2[:])

            # ---------------- main schedule ----------------
            phases = os.environ.get("KPHASES", "all")
            if phases != "all":
                nc.gpsimd.memset(densC[:], 0.0)
                nc.gpsimd.memset(curT[1][:], 0.0)
                nc.gpsimd.memset(curT[2][:], 0.0)

            def layer(l, with_gat):
                stats_l, xsall = stats_phase(l)
                gen = None
                if with_gat:
                    gat_prologue(l)
                    gen = gat_edges(l, chunk_tile)
                nbatch_total = (C + BATCH - 1) // BATCH
                emitted = 0
                # interleave: preps up front (4 independent chains), then
                # 4x16 slots; emit edge batches between slots so GAT Pool/DMA
                # work overlaps readout compute
                points = 4 + 4 * 16
                per_point = nbatch_total / points if with_gat else 0.0
                acc = 0.0

                def drain():
                    nonlocal emitted, acc
                    acc += per_point
                    while gen is not None and emitted < min(nbatch_total, int(round(acc))):
                        try:
                            next(gen); emitted += 1
                        except StopIteration:
                            return

                preps = []
                for g in range(G):
                    cstack_g, xh_s_g, xsq_g = derive_graph(l, g, xsall, *stats_l[g])
                    preps.append(unit_prep(l, g, cstack_g, xh_s_g, xsq_g))
                    drain()
                for g in range(G):
                    u = l * G + g
                    Lq, X4 = preps[g]
                    for k in range(DPC):
                        slot(u, k, Lq, X4)
                        drain()
                if with_gat:
                    for _ in gen:
                        pass

            for _ in range(reps):
                if phases == "r0":
                    layer(0, False)
                    quantiles(0, 64)
                elif phases == "r0g0":
                    layer(0, True)
                    quantiles(0, 64)
                else:
                    layer(0, True)
                    quantiles(0, 64)
                    layer(1, True)
                    quantiles(64, 128)
                    layer(2, False)
                    quantiles(128, 192)

            if KDBG:
                nc.sync.dma_start(out=dbg_dens[:], in_=densC[:])
            nc.sync.dma_start(out=pmean_out[:], in_=pmean[:])
            nc.sync.dma_start(out=pmax_out[:], in_=pmax[:])
    nc.compile()
    return nc


_CACHE = {}


def _get_program(C, chunk_tile, reps=1):
    key = (C, tuple(chunk_tile.tolist()), reps,
           os.environ.get("KPHASES", "all"), os.environ.get("KDEBUG"))
    if key not in _CACHE:
        _CACHE[key] = build_program(C, chunk_tile, reps)
    return _CACHE[key]


def _host_inputs(inputs, s_idx, d_idx, dlf, maskneg, C):
    x = np.asarray(inputs["x"], np.float32)
    repq = np.zeros((DPC, 3, 512), np.float32)
    repx = np.zeros((DPC, 4, 3, 128), np.float32)
    for k in range(DPC):
        m, s = divmod(k, 4)
        for r in range(3):
            repq[k, r, 128 * m + 32 * s + r] = 1.0
            repx[k, m, r, 32 * s + r] = -0.5 if r == 2 else 1.0
    import ml_dtypes
    kk = np.arange(GCP, dtype=np.float64) - (GC - 1) / 2.0

    def wrap16(idx):
        # idx [128, C] int32 -> [128, CW] i16: global edge j=c*128+e at [j%16, j//16],
        # replicated across the 8 Q7 cores (partition blocks of 16)
        Cn = idx.shape[1]
        CW = ((Cn + BATCH - 1) // BATCH) * BATCH * 8
        flat = idx.T.ravel()                       # j = c*128+e order
        t = np.zeros((16, CW), np.int16)
        jj = np.arange(Cn * 128)
        t[jj % 16, jj // 16] = flat.astype(np.int16)
        return np.tile(t, (8, 1))

    # fused interp+cumsum matrix: cdf500 = densC^T @ triM
    M = np.zeros((GC, GRID))
    pos = np.arange(GRID) * (GC - 1) / (GRID - 1)
    lo = np.floor(pos).astype(int)
    wf = pos - lo
    hi = np.minimum(lo + 1, GC - 1)
    np.add.at(M, (lo, np.arange(GRID)), 1 - wf)
    np.add.at(M, (hi, np.arange(GRID)), wf)
    triM = np.zeros((GC, 512), np.float32)
    triM[:, 0:GRID] = np.cumsum(M, axis=1)

    oiT = np.zeros((128, 2, 4), np.float32)
    oiT[:, 0, :] = 1.0
    oiT[:, 1, :] = (np.arange(128, dtype=np.float32)[:, None]
                    + 128.0 * np.arange(4, dtype=np.float32)[None, :])

    qsv = np.linspace(0.0, 1.0, Q)
    qbias = np.zeros((128, 2 * Q + 1), np.float32)
    qbias[:, 0:2 * Q:2] = 100.0 * qsv[None, :]
    qbias[:, 1:2 * Q:2] = -100.0 * qsv[None, :]
    qbias[:, 2 * Q] = 1e-8

    im_base = dict(
        repq=repq, repx=repx,
        xT=np.ascontiguousarray(x.T),
        sidx=wrap16(s_idx), didx=wrap16(d_idx),
        dlfb=dlf.astype(ml_dtypes.bfloat16),
        mneg=maskneg,
        iota3=np.stack([np.ones(GCP), kk, kk ** 2]).astype(np.float32),
        triM=triM,
        oiT=oiT,
        qbias=qbias,
        iotaF=np.tile(np.arange(128, dtype=np.float32)[None, :], (128, 1)).astype(ml_dtypes.bfloat16),
    )
    for l in range(2):
        A = np.zeros((128, 8), np.float32)
        as_l = np.asarray(inputs[f"as{l}"], np.float32)
        ad_l = np.asarray(inputs[f"ad{l}"], np.float32)
        for h in range(HEADS):
            A[h * HID:(h + 1) * HID, h] = as_l[h]
            A[h * HID:(h + 1) * HID, 4 + h] = ad_l[h]
        im_base[f"W{l}"] = np.asarray(inputs[f"W{l}"], np.float32)
        im_base[f"Av{l}"] = A
        im_base[f"bcol{l}"] = np.asarray(inputs[f"b{l}"], np.float32).reshape(128, 1)
    in_maps = []
    for c in range(N_CORES):
        selm = np.zeros((128, DPC), np.float32)
        for k in range(DPC):
            selm[DPC * c + k, k] = 1.0
        in_maps.append({**im_base, "sel": selm})
    return in_maps


def kernel(**inputs) -> np.ndarray:
    from concourse.bass_utils import run_bass_kernel_spmd
    s_idx, d_idx, dlf, maskneg, chunk_tile = _edge_prep(np.asarray(inputs["edge_index"]))
    C = s_idx.shape[1]
    nc = _get_program(C, chunk_tile)
    in_maps = _host_inputs(inputs, s_idx, d_idx, dlf, maskneg, C)
    res = run_bass_kernel_spmd(nc, in_maps, list(range(N_CORES))).results
    return _assemble(inputs, res)


def _assemble(inputs, res):
    # kf[l, g, d, q]
    kf = np.zeros((3, G, 128, Q), np.float64)
    for c in range(N_CORES):
        s0t1 = np.asarray(res[c]["s0t1"], np.float64)    # [2Q, 192]
        S0, T1 = s0t1[0::2, :], s0t1[1::2, :]            # [Q, 192]
        mndl2 = np.asarray(res[c]["mndl2"], np.float64)  # [16, 24]
        mn = mndl2[:, 0::2].T.reshape(-1)                # [192] slot-ordered
        dl = mndl2[:, 1::2].T.reshape(-1)
        qvT = (mn[None, :] * S0 + dl[None, :] * T1) / (S0 + 1e-8)
        for u in range(UNITS):
            l, g = divmod(u, G)
            kf[l, g, DPC * c:DPC * (c + 1), :] = qvT[:, u * DPC:(u + 1) * DPC].T
    pmean = res[0]["pmean"] / 512.0          # [128, 12]
    pmax = res[0]["pmax"]
    pool_w = np.asarray(inputs["pool_w"], np.float64)
    beta = np.asarray(inputs["beta"], np.float64)
    h0 = float(np.asarray(inputs["h0"]).reshape(-1)[0])
    h_list, k_list = [], []
    for l in range(3):
        wp = (pool_w[0] * pmean[:, l * G:(l + 1) * G] + pool_w[1] * pmax[:, l * G:(l + 1) * G]).T
        lpW = np.asarray(inputs[f"lpW{l}"], np.float64)
        lpb = np.asarray(inputs[f"lpb{l}"], np.float64)
        h_list.append(wp @ lpW + lpb)
        kW = np.asarray(inputs[f"kW{l}"], np.float64)
        kb = np.asarray(inputs[f"kb{l}"], np.float64)
        k_list.append(kf[l].reshape(G, 128 * Q) @ kW + kb)
    main_out = np.mean(h_list, axis=0)
    kde_out = np.mean(k_list, axis=0)
    risk = (main_out + kde_out) @ beta + h0
    return risk.astype(np.float32)


# revision 53
# speedup vs baseline: 1.1306x; 1.0453x over previous
"""Trainium2 Bass kernel for nn_GAT_KDE_14766097563859.

2-layer GAT over a 2048-node graph + per-(graph,layer) KDE soft-quantile
readouts. SPMD over 8 NeuronCores: GAT replicated, KDE sharded by feature dim
(each core owns 16 of 128 dims for all 12 (layer,graph) units = 192 slots).

v3 (over the v2 baseline):
- density evaluated on a 125-point coarse grid; cumsum+linear-interp back to
  the 500-point cdf is folded into one constant [125,500] matmul matrix
  (linear maps compose), cutting exp/matmul volume 4x
- per-slot node-sum fused into the exp activation via accum_out (zero DVE)
- quantile stage flipped to [fine-grid part, slot free] orientation:
  sigma(-100|d|) = min(sigma(-100d), sigma(100d)) with the q-shift folded
  into the sigmoid bias; S0/T1 reductions over the grid run on PE
- PSUM->SBUF copies moved to Act; xh_s/xsq on Act (scale-AP / Square, with
  the -0.5 folded into the repx constant); per-graph xs kept, not re-derived
"""
import os
import sys
sys.path.insert(0, "/opt/trn_rl_repo")
import numpy as np

import concourse.bass as bass
import concourse.bacc as bacc
import concourse.mybir as mybir
from concourse.tile import TileContext
from concourse.masks import make_identity
from concourse import library_config

F32 = mybir.dt.float32
F32R = mybir.dt.float32r
BF16 = mybir.dt.bfloat16
I32 = mybir.dt.int32
I16 = mybir.dt.int16
AF = mybir.ActivationFunctionType
ALU = mybir.AluOpType
AX = mybir.AxisListType

G, NG, N, E = 4, 512, 2048, 32768
IN_DIM, HID, HEADS, HC, OUT_DIM = 128, 32, 4, 128, 32
N_LAYERS, GRID, Q, NEG_SLOPE = 2, 500, 20, 0.2
N_CORES = 8
UNITS = (N_LAYERS + 1) * G            # 12, unit u = l*G + g
DPC = 16                              # dims per core
SLOTS = UNITS * DPC                   # 192
GEFF = 500                            # fine grid (matches reference GRID)
GC = 125                              # coarse density grid
GCP = 128                             # GC padded (fp32r matmul needs free%4==0)
BATCH = 8                             # chunks per gather batch (1024 idxs = SWDGE ring cap)
TBLW = 256                            # bf16 cols of xh_al row (512B, dma_gather needs %256B)
ALDW = 128                            # bf16 cols of al_d row (256B)


def _edge_prep(edge_index):
    src = edge_index[0].astype(np.int64)
    dst = edge_index[1].astype(np.int64)
    s_all = np.concatenate([src, np.arange(N)])
    d_all = np.concatenate([dst, np.arange(N)])
    order = np.argsort(d_all, kind="stable")
    s_s, d_s = s_all[order], d_all[order]
    cs, cd, ct, cm = [], [], [], []
    for t in range(16):
        sel = (d_s // 128) == t
        se, de = s_s[sel], d_s[sel]
        ne = len(se)
        npad = (-ne) % 128
        se = np.concatenate([se, np.zeros(npad, np.int64)])
        de = np.concatenate([de, np.full(npad, t * 128, np.int64)])
        rm = np.concatenate([np.ones(ne, bool), np.zeros(npad, bool)])
        for c0 in range(0, len(se), 128):
            cs.append(se[c0:c0 + 128]); cd.append(de[c0:c0 + 128])
            cm.append(rm[c0:c0 + 128]); ct.append(t)
    C = len(cs)
    s_idx = np.stack(cs, 1).astype(np.int32)                  # [128, C] src ids
    d_idx = np.stack(cd, 1).astype(np.int32)                  # [128, C] dst ids
    dlf = (np.stack(cd, 1) - np.asarray(ct)[None, :] * 128).astype(np.float32)
    maskneg = np.stack(cm, 1).astype(np.float32)  # 1 valid, 0 pad
    return s_idx, d_idx, dlf, maskneg, np.asarray(ct, np.int32)


def build_program(C, chunk_tile, reps=1):
    nc = bacc.Bacc(None, target_bir_lowering=False, debug=True)

    xT_in = nc.declare_dram_parameter("xT", [128, N], F32, isOutput=False)
    Wi = [nc.declare_dram_parameter(f"W{l}", [128, HC], F32, isOutput=False) for l in range(2)]
    Avi = [nc.declare_dram_parameter(f"Av{l}", [128, 8], F32, isOutput=False) for l in range(2)]
    bci = [nc.declare_dram_parameter(f"bcol{l}", [128, 1], F32, isOutput=False) for l in range(2)]
    sel_in = nc.declare_dram_parameter("sel", [128, DPC], F32, isOutput=False)
    iota3_in = nc.declare_dram_parameter("iota3", [3, GCP], F32R, isOutput=False)
    triM_in = nc.declare_dram_parameter("triM", [GC, 512], F32, isOutput=False)
    oiT_in = nc.declare_dram_parameter("oiT", [128, 2, 4], F32, isOutput=False)
    qbias_in = nc.declare_dram_parameter("qbias", [128, 2 * Q + 1], F32, isOutput=False)
    iotaF_in = nc.declare_dram_parameter("iotaF", [128, 128], BF16, isOutput=False)
    CW = ((C + BATCH - 1) // BATCH) * BATCH * 8
    sidx_in = nc.declare_dram_parameter("sidx", [128, CW], I16, isOutput=False)
    didx_in = nc.declare_dram_parameter("didx", [128, CW], I16, isOutput=False)
    dlfb_in = nc.declare_dram_parameter("dlfb", [128, C], BF16, isOutput=False)
    mneg_in = nc.declare_dram_parameter("mneg", [128, C], F32, isOutput=False)
    repq_in = nc.declare_dram_parameter("repq", [DPC, 3, 512], F32R, isOutput=False)
    repx_in = nc.declare_dram_parameter("repx", [DPC, 4, 3, 128], F32R, isOutput=False)

    s0t1_out = nc.declare_dram_parameter("s0t1", [2 * Q, SLOTS], F32, isOutput=True)
    mndl_out = nc.declare_dram_parameter("mndl2", [DPC, 2 * UNITS], F32, isOutput=True)
    pmean_out = nc.declare_dram_parameter("pmean", [128, UNITS], F32, isOutput=True)
    pmax_out = nc.declare_dram_parameter("pmax", [128, UNITS], F32, isOutput=True)
    KDBG = os.environ.get("KDEBUG") == "1"
    if KDBG:
        dbg_dens = nc.declare_dram_parameter("dbg_dens", [GC, SLOTS], F32, isOutput=True)

    xh_al_hbm = nc.dram_tensor("xh_al", [N, TBLW], BF16)
    ald_hbm = nc.dram_tensor("ald", [N, ALDW], BF16)

    with TileContext(nc) as tc:
        with (
            tc.tile_pool(name="cst", bufs=1) as cst,
            tc.tile_pool(name="wrk", bufs=2) as sb,
            tc.tile_pool(name="ps", bufs=1, space="PSUM") as ps,
        ):
            # ---------------- resident state ----------------
            curT = [cst.tile([128, N], F32, tag=f"curT{l}", name=f"curT{l}") for l in range(3)]
            W = [cst.tile([128, HC], F32, tag=f"Wt{l}", name=f"Wt{l}") for l in range(2)]
            Av = [cst.tile([128, 8], F32, tag=f"Avt{l}", name=f"Avt{l}") for l in range(2)]
            bcol = [cst.tile([128, 1], F32, tag=f"bct{l}", name=f"bct{l}") for l in range(2)]
            sel = cst.tile([128, DPC], F32)
            iota3 = cst.tile([3, GCP], F32R)
            triM = cst.tile([GC, 512], F32)
            oiT = cst.tile([128, 2, 4], F32)
            qbias = cst.tile([128, 2 * Q + 1], F32)
            iotaF = cst.tile([128, 128], BF16)
            sidx = cst.tile([128, CW], I16)
            didx = cst.tile([128, CW], I16)
            dlfb = cst.tile([128, C], BF16)
            mneg = cst.tile([128, C], F32)
            repq = cst.tile([DPC, 3, 512], F32R)
            repx = cst.tile([DPC, 4, 3, 128], F32R)
            ones16 = cst.tile([DPC, 512], F32R)
            onesrow = cst.tile([1, 128], F32)
            densC = cst.tile([GC, SLOTS], F32)
            pmean = cst.tile([128, UNITS], F32)
            pmax = cst.tile([128, UNITS], F32)
            id128 = cst.tile([128, 128], F32)
            xhT = cst.tile([128, N], F32)
            al = cst.tile([128, 16, 8], F32)
            stage = cst.tile([128, 16, TBLW], BF16)
            stage_d = cst.tile([128, 16, ALDW], BF16)
            nc.gpsimd.load_library(library_config.mlp)
            make_identity(nc, id128[:])
            nc.gpsimd.dma_start(out=curT[0][:], in_=xT_in[:])
            for l in range(2):
                nc.gpsimd.dma_start(out=W[l][:], in_=Wi[l][:])
                nc.gpsimd.dma_start(out=Av[l][:], in_=Avi[l][:])
                nc.gpsimd.dma_start(out=bcol[l][:], in_=bci[l][:])
            nc.gpsimd.dma_start(out=sel[:], in_=sel_in[:])
            nc.gpsimd.dma_start(out=iota3[:], in_=iota3_in[:])
            nc.gpsimd.dma_start(out=triM[:], in_=triM_in[:])
            nc.gpsimd.dma_start(out=oiT[:], in_=oiT_in[:])
            nc.gpsimd.dma_start(out=qbias[:], in_=qbias_in[:])
            nc.gpsimd.dma_start(out=iotaF[:], in_=iotaF_in[:])
            nc.gpsimd.dma_start(out=sidx[:], in_=sidx_in[:])
            nc.gpsimd.dma_start(out=didx[:], in_=didx_in[:])
            nc.gpsimd.dma_start(out=dlfb[:], in_=dlfb_in[:])
            nc.gpsimd.dma_start(out=mneg[:], in_=mneg_in[:])
            nc.gpsimd.dma_start(out=repq[:], in_=repq_in[:])
            nc.gpsimd.dma_start(out=repx[:], in_=repx_in[:])
            nc.gpsimd.memset(ones16[:].bitcast(F32), 1.0)
            nc.gpsimd.memset(onesrow[:], 1.0)
            nc.gpsimd.memset(stage[:].bitcast(mybir.dt.uint16), 0)
            nc.gpsimd.memset(stage_d[:].bitcast(mybir.dt.uint16), 0)
            # constant 1.0 columns of the gather table (col 33h+32), set once
            nc.gpsimd.memset(stage[:, :, 0:132].rearrange("p t (h c) -> p t h c", c=33)[:, :, :, 32:33].bitcast(mybir.dt.uint16), 0x3F80)

            # ---------------- per-layer stats (all 4 graphs) ----------------
            def stats_phase(l):
                cur = curT[l]
                xsall = sb.tile([DPC, G, 512], F32, tag="xsall", bufs=2)
                per_g = []
                for g in range(G):
                    u = l * G + g
                    blk = cur[:, 512 * g:512 * (g + 1)]
                    nc.vector.tensor_reduce(out=pmean[:, u:u + 1], in_=blk, axis=AX.X, op=ALU.add)
                    nc.vector.tensor_reduce(out=pmax[:, u:u + 1], in_=blk, axis=AX.X, op=ALU.max)
                    ps_xs = ps.tile([DPC, 512], F32, tag="ps_m", bufs=2)
                    nc.tensor.matmul(ps_xs[:], sel[:], blk, start=True, stop=True)
                    xs = xsall[:, g, :]
                    nc.scalar.activation(xs, ps_xs[:], AF.Identity)
                    mn = sb.tile([DPC, 1], F32, tag=f"mn{g}", name=f"mn{g}", bufs=2)
                    mx = sb.tile([DPC, 1], F32, tag="mx")
                    sm = sb.tile([DPC, 1], F32, tag="sm")
                    sq = sb.tile([DPC, 1], F32, tag="sq")
                    nc.vector.tensor_reduce(out=mn[:], in_=xs, axis=AX.X, op=ALU.min)
                    nc.vector.tensor_reduce(out=mx[:], in_=xs, axis=AX.X, op=ALU.max)
                    nc.vector.tensor_reduce(out=sm[:], in_=xs, axis=AX.X, op=ALU.add)
                    xsq0 = sb.tile([DPC, 512], F32, tag="xsq0")
                    nc.vector.tensor_tensor(out=xsq0[:], in0=xs, in1=xs, op=ALU.mult)
                    nc.vector.tensor_reduce(out=sq[:], in_=xsq0[:], axis=AX.X, op=ALU.add)
                    var = sb.tile([DPC, 1], F32, tag=f"var{g}", name=f"var{g}", bufs=2)
                    mean = sb.tile([DPC, 1], F32, tag="mean")
                    nc.vector.tensor_scalar(out=mean[:], in0=sm[:], scalar1=1.0 / 512,
                                            scalar2=None, op0=ALU.mult)
                    nc.vector.tensor_scalar(out=var[:], in0=sq[:], scalar1=1.0 / 512,
                                            scalar2=None, op0=ALU.mult)
                    m2 = sb.tile([DPC, 1], F32, tag="m2")
                    nc.vector.tensor_tensor(out=m2[:], in0=mean[:], in1=mean[:], op=ALU.mult)
                    nc.vector.tensor_tensor(out=var[:], in0=var[:], in1=m2[:], op=ALU.subtract)
                    nc.vector.tensor_scalar(out=mn[:], in0=mn[:], scalar1=-1e-6, scalar2=None, op0=ALU.add)
                    nc.vector.tensor_scalar(out=mx[:], in0=mx[:], scalar1=1e-6, scalar2=None, op0=ALU.add)
                    dl = sb.tile([DPC, 1], F32, tag=f"dl{g}", name=f"dl{g}", bufs=2)
                    nc.vector.tensor_tensor(out=dl[:], in0=mx[:], in1=mn[:], op=ALU.subtract)
                    nc.vector.tensor_scalar(out=dl[:], in0=dl[:], scalar1=1.0 / (GEFF - 1), scalar2=None, op0=ALU.mult)
                    per_g.append((mn, dl, var))
                # std = sqrt(var) on DVE (rsqrt bit-trick + 3 Newton steps)
                # so Act never loads the sqrt table (avoids table thrash
                # against the overlapped quantile-pass sigmoids)
                out = []
                for g, (mn, dl, var) in enumerate(per_g):
                    u = l * G + g
                    y = sb.tile([DPC, 1], F32, tag="rsq_y", bufs=2)
                    nc.vector.tensor_scalar(out=y[:].bitcast(I32), in0=var[:].bitcast(I32),
                                            scalar1=1, scalar2=None,
                                            op0=ALU.logical_shift_right)
                    nc.vector.tensor_scalar(out=y[:].bitcast(I32), in0=y[:].bitcast(I32),
                                            scalar1=-1, scalar2=None,
                                            op0=ALU.bitwise_xor)
                    nc.vector.tensor_scalar(out=y[:].bitcast(I32), in0=y[:].bitcast(I32),
                                            scalar1=0x5F3759E0, scalar2=None,
                                            op0=ALU.add)
                    vh = sb.tile([DPC, 1], F32, tag="rsq_vh", bufs=2)
                    nc.vector.tensor_scalar(out=vh[:], in0=var[:], scalar1=-0.5,
                                            scalar2=None, op0=ALU.mult)
                    for _ in range(3):
                        y2 = sb.tile([DPC, 1], F32, tag="rsq_y2", bufs=2)
                        nc.vector.tensor_tensor(out=y2[:], in0=y[:], in1=y[:], op=ALU.mult)
                        nc.vector.scalar_tensor_tensor(out=y2[:], in0=y2[:], scalar=1.0,
                                                       in1=vh[:], op0=ALU.mult,
                                                       op1=ALU.mult)
                        nc.vector.tensor_scalar(out=y2[:], in0=y2[:], scalar1=1.5,
                                                scalar2=None, op0=ALU.add)
                        nc.vector.tensor_tensor(out=y[:], in0=y[:], in1=y2[:], op=ALU.mult)
                    std = sb.tile([DPC, 1], F32, tag=f"std{g}", name=f"std{g}", bufs=2)
                    nc.vector.tensor_tensor(out=std[:], in0=var[:], in1=y[:], op=ALU.mult)
                    nc.sync.dma_start(out=mndl_out[:, 2 * u:2 * u + 1], in_=mn[:])
                    nc.sync.dma_start(out=mndl_out[:, 2 * u + 1:2 * u + 2], in_=dl[:])
                    out.append((mn, dl, std))
                return out, xsall

            def derive_graph(l, g, xsall, mn, dl, std):
                """Per-graph slot-pass prep: build cstack + X rows from kept xs."""
                xs = xsall[:, g, :]
                h = sb.tile([DPC, 1], F32, tag="h")
                nc.vector.tensor_scalar(out=h[:], in0=std[:], scalar1=float(1e-8 / 3),
                                        scalar2=float(1.06 * 512 ** -0.2),
                                        op0=ALU.add, op1=ALU.mult)
                rh = sb.tile([DPC, 1], F32, tag="rh", bufs=2)
                nc.vector.reciprocal(out=rh[:], in_=h[:])
                # recentered basis: k' = k-(GC-1)/2, x' = (x-c)/h with c the
                # grid midpoint -> a0 = 0 and all poly terms stay O(span/2h),
                # minimizing f32r (tf32) rounding of the stored poly values
                a0 = sb.tile([DPC, 1], F32, tag="a0")
                a1 = sb.tile([DPC, 1], F32, tag="a1")
                nc.gpsimd.memset(a0[:], 0.0)
                # coarse grid step is (GEFF-1)/(GC-1) fine steps
                nc.vector.scalar_tensor_tensor(
                    out=a1[:], in0=dl[:], scalar=float((GEFF - 1) / (GC - 1)),
                    in1=rh[:], op0=ALU.mult, op1=ALU.mult)
                mnrh = sb.tile([DPC, 1], F32, tag="mnrh")
                nc.vector.tensor_tensor(out=mnrh[:], in0=mn[:], in1=rh[:], op=ALU.mult)
                biasc = sb.tile([DPC, 1], F32, tag="biasc", bufs=2)
                nc.vector.scalar_tensor_tensor(
                    out=biasc[:], in0=a1[:], scalar=-float((GC - 1) / 2),
                    in1=mnrh[:], op0=ALU.mult, op1=ALU.subtract)
                # cstack [16, 9]: r0=(q0,q1,q2) r1=(a0,a1,0) r2=(1,0,0)
                cstack = sb.tile([DPC, 9], F32R, tag="cs", bufs=2)
                nc.gpsimd.memset(cstack[:].bitcast(F32), 0.0)
                nc.vector.tensor_tensor(out=cstack[:, 0:1], in0=a0[:], in1=a0[:], op=ALU.mult)
                nc.vector.tensor_scalar(out=cstack[:, 0:1], in0=cstack[:, 0:1], scalar1=-0.5, scalar2=None, op0=ALU.mult)
                nc.vector.tensor_tensor(out=cstack[:, 1:2], in0=a0[:], in1=a1[:], op=ALU.mult)
                nc.vector.tensor_scalar(out=cstack[:, 1:2], in0=cstack[:, 1:2], scalar1=-1.0, scalar2=None, op0=ALU.mult)
                nc.vector.tensor_tensor(out=cstack[:, 2:3], in0=a1[:], in1=a1[:], op=ALU.mult)
                nc.vector.tensor_scalar(out=cstack[:, 2:3], in0=cstack[:, 2:3], scalar1=-0.5, scalar2=None, op0=ALU.mult)
                nc.vector.tensor_copy(cstack[:, 3:4], a0[:])
                nc.vector.tensor_copy(cstack[:, 4:5], a1[:])
                nc.gpsimd.memset(cstack[:, 6:7].bitcast(F32), 1.0)
                xh_s = sb.tile([DPC, 512], F32R, tag="xh_s", bufs=2)
                nc.scalar.activation(xh_s[:], xs, AF.Identity, scale=rh[:, 0:1],
                                     bias=biasc[:, 0:1])
                xsq = sb.tile([DPC, 512], F32R, tag="xsq", bufs=2)
                nc.scalar.activation(xsq[:], xh_s[:], AF.Square)
                return cstack, xh_s, xsq

            # ---------------- per-(unit) prep: Lq / X4 ----------------
            def unit_prep(l, g, cstack_g, xh_s_g, xsq_g):
                # lh [3, 512]: col 128m+32s+r' = cstack triple r' of slot k=4m+s
                ps_lh = ps.tile([3, 512], F32, tag="ps_m", bufs=2)
                for r in range(3):
                    nc.tensor.matmul(ps_lh[:], cstack_g[:, 3 * r:3 * r + 3],
                                     repq[:, r, :],
                                     start=(r == 0), stop=(r == 2),
                                     skip_group_check=True)
                lh = sb.tile([3, 512], F32R, tag="lh", bufs=2)
                nc.scalar.activation(lh[:], ps_lh[:], AF.Identity)
                Lq = []
                X4 = []
                for m in range(4):
                    ps_L = ps.tile([128, GCP], F32, tag="ps_m", bufs=2)
                    nc.tensor.matmul(ps_L[:], lh[:, 128 * m:128 * (m + 1)], iota3[:],
                                     start=True, stop=True)
                    Lqm = sb.tile([128, GCP], F32R, tag=f"Lq{m}", name=f"Lqm{m}", bufs=4)
                    nc.scalar.activation(Lqm[:], ps_L[:], AF.Identity)
                    Lq.append(Lqm)
                    ps_X = ps.tile([128, 512], F32, tag="ps_m", bufs=2)
                    nc.tensor.matmul(ps_X[:], repx[:, m, 0, :],
                                     ones16[:], start=True, stop=False, skip_group_check=True)
                    nc.tensor.matmul(ps_X[:], repx[:, m, 1, :], xh_s_g,
                                     start=False, stop=False, skip_group_check=True)
                    # repx r=2 entries are -0.5: folds the -u^2/2 scaling
                    nc.tensor.matmul(ps_X[:], repx[:, m, 2, :], xsq_g,
                                     start=False, stop=True, skip_group_check=True)
                    X4m = sb.tile([128, 512], F32R, tag=f"X4{m}", name=f"X4m{m}", bufs=4)
                    nc.scalar.activation(X4m[:], ps_X[:], AF.Identity)
                    X4.append(X4m)
                return Lq, X4

            # ---------------- one KDE slot ----------------
            def slot(u, k, Lq, X4):
                m, s = divmod(k, 4)
                sidx_ = u * DPC + k
                psu = ps.tile([GC, 512], F32, tag="psu", bufs=2)
                nc.tensor.matmul(psu[:], Lq[m][32 * s:32 * s + 3, 0:GC],
                                 X4[m][32 * s:32 * s + 3, :], start=True, stop=True,
                                 tile_position=(32 * s, 0), skip_group_check=True)
                dump = sb.tile([GC, 512], BF16, tag="dump", bufs=2)
                nc.scalar.activation(dump[:], psu[:], AF.Exp,
                                     accum_out=densC[:, sidx_:sidx_ + 1])

            # ---------------- GAT prologue ----------------
            def gat_prologue(l):
                cur = curT[l]
                for b in range(4):
                    pxh = ps.tile([128, 512], F32, tag="ps_m", bufs=2)
                    nc.tensor.matmul(pxh[:], W[l][:], cur[:, 512 * b:512 * (b + 1)],
                                     start=True, stop=True)
                    nc.vector.tensor_copy(xhT[:, 512 * b:512 * (b + 1)], pxh[:])
                for t in range(16):
                    pal = ps.tile([128, 8], F32, tag="ps_m", bufs=2)
                    nc.tensor.matmul(pal[:], xhT[:, 128 * t:128 * (t + 1)], Av[l][:],
                                     start=True, stop=True)
                    nc.vector.tensor_copy(al[:, t, :], pal[:])
                    pxr = ps.tile([128, 128], F32, tag="ps_m", bufs=2)
                    nc.tensor.transpose(pxr[:], xhT[:, 128 * t:128 * (t + 1)], id128[:])
                    nc.vector.tensor_copy(
                        stage[:, t, 0:132].rearrange("p (h c) -> p h c", h=4)[:, :, 0:32],
                        pxr[:].rearrange("p (h c) -> p h c", h=4))
                # al_s -> bf16 table cols 132:140 viewed as f32 x4
                nc.vector.tensor_copy(stage[:, :, 132:140].bitcast(F32), al[:, :, 0:4])
                nc.vector.tensor_copy(stage_d[:, :, 0:8].bitcast(F32), al[:, :, 4:8])
                nc.sync.dma_start(out=xh_al_hbm[:].rearrange("(t p) d -> p t d", p=128),
                                  in_=stage[:])
                nc.sync.dma_start(out=ald_hbm[:].rearrange("(t p) d -> p t d", p=128),
                                  in_=stage_d[:])

            # ---------------- GAT edge batches ----------------
            def finish_tile(l, t, raw):
                rawv = raw[:].rearrange("p (h c) -> p h c", h=4)
                rd = sb.tile([128, 4], F32, tag="rd")
                nc.vector.tensor_scalar(out=rd[:], in0=rawv[:, :, 32], scalar1=1e-16,
                                        scalar2=None, op0=ALU.add)
                nc.vector.reciprocal(out=rd[:], in_=rd[:])
                o = sb.tile([128, HC], F32, tag="otile", bufs=2)
                nc.vector.tensor_tensor(
                    out=o[:].rearrange("p (h c2) -> p h c2", h=4),
                    in0=rawv[:, :, 0:32],
                    in1=rd[:].rearrange("p h -> p h ()").to_broadcast([128, 4, 32]),
                    op=ALU.mult)
                pt = ps.tile([128, 128], F32, tag="ps_m", bufs=2)
                nc.tensor.transpose(pt[:], o[:], id128[:])
                nc.scalar.activation(curT[l + 1][:, 128 * t:128 * (t + 1)], pt[:],
                                     AF.Relu if l == 0 else AF.Identity,
                                     bias=bcol[l][:, 0:1])

            def gat_edges(l, chunk_tile):
                """Generator: yields after each emitted batch."""
                raw = None
                cur_t = -1
                for b0 in range(0, C, BATCH):
                    cn = min(BATCH, C - b0)
                    gwin = sb.tile([128, BATCH, TBLW], BF16, tag="gwin", bufs=2)
                    nc.gpsimd.dma_gather(gwin[:], xh_al_hbm[:],
                                         sidx[:, b0 * 8:(b0 + BATCH) * 8],
                                         BATCH * 128, BATCH * 128, TBLW, queue_num=0)
                    aldw = sb.tile([128, BATCH, ALDW], BF16, tag="aldw", bufs=2)
                    nc.gpsimd.dma_gather(aldw[:], ald_hbm[:],
                                         didx[:, b0 * 8:(b0 + BATCH) * 8],
                                         BATCH * 128, BATCH * 128, ALDW, queue_num=0)
                    # z = al_s[src] + al_d[dst]; leaky-relu; +maskneg; exp -> bf16
                    z = sb.tile([128, BATCH, 4], F32, tag="z", bufs=2)
                    nc.vector.tensor_tensor(out=z[:, 0:cn, :],
                                            in0=gwin[:, 0:cn, 132:140].bitcast(F32),
                                            in1=aldw[:, 0:cn, 0:8].bitcast(F32), op=ALU.add)
                    # leaky-relu fused: max(z, 0.2*z)
                    zl = sb.tile([128, BATCH, 4], F32, tag="zl", bufs=2)
                    nc.vector.scalar_tensor_tensor(
                        out=zl[:, 0:cn, :], in0=z[:, 0:cn, :], scalar=NEG_SLOPE,
                        in1=z[:, 0:cn, :], op0=ALU.mult, op1=ALU.max)
                    p_r = sb.tile([128, BATCH, 4], BF16, tag="p_r", bufs=2)
                    nc.scalar.activation(p_r[:, 0:cn, :], zl[:, 0:cn, :], AF.Exp)
                    nc.vector.tensor_tensor(
                        out=p_r[:, 0:cn, :], in0=p_r[:, 0:cn, :],
                        in1=mneg[:, b0:b0 + cn].rearrange("p c -> p c ()").to_broadcast([128, cn, 4]),
                        op=ALU.mult)
                    # one-hot [e, d] for the whole batch
                    OH = sb.tile([128, BATCH, 128], BF16, tag="OH", bufs=2)
                    nc.vector.tensor_tensor(
                        out=OH[:, 0:cn, :],
                        in0=dlfb[:, b0:b0 + cn].rearrange("p c -> p c ()").to_broadcast([128, cn, 128]),
                        in1=iotaF[:].rearrange("p f -> p () f").to_broadcast([128, cn, 128]),
                        op=ALU.is_equal)
                    # sxh = gathered (feat|1.0) * alpha  (4x33 interleave)
                    sxh = sb.tile([128, BATCH, 132], BF16, tag="sxh", bufs=2)
                    nc.vector.tensor_tensor(
                        out=sxh[:, 0:cn, :].rearrange("p b (h c) -> p b h c", h=4),
                        in0=gwin[:, 0:cn, 0:132].rearrange("p b (h c) -> p b h c", h=4),
                        in1=p_r[:, 0:cn, :].rearrange("p b h -> p b h ()").to_broadcast([128, cn, 4, 33]),
                        op=ALU.mult)
                    for ci in range(cn):
                        c = b0 + ci
                        t = int(chunk_tile[c])
                        first = (c == 0) or (int(chunk_tile[c - 1]) != t)
                        last = (c == C - 1) or (int(chunk_tile[c + 1]) != t)
                        if first:
                            if raw is not None:
                                finish_tile(l, cur_t, raw)
                            raw = ps.tile([128, 132], F32, tag="raw",
                                          padded_shape=[128, 512], bufs=2)
                            cur_t = t
                        nc.tensor.matmul(raw[:], OH[:, ci, :], sxh[:, ci, :],
                                         start=first, stop=last, skip_group_check=True)
                    yield
                if raw is not None:
                    finish_tile(l, cur_t, raw)

            # ---------------- quantile stage (per-layer slice pass) ----------------
            def quantiles(s0, s1):
                qs = np.linspace(0.0, 1.0, Q)
                tws = [min(128, GEFF - 128 * t) for t in range(4)]
                SW = s1 - s0
                dC = densC[:, s0:s1]
                # cdf at fine grid, [fine part (4x128), slot-slice free] via PE
                cdfp = sb.tile([128, 4, SW], F32, tag="cdfp", bufs=2)
                for t in range(4):
                    cps = ps.tile([128, 512], F32, tag="ps_m", bufs=2)
                    nc.tensor.matmul(cps[:, 0:SW], triM[:, 128 * t:128 * (t + 1)],
                                     dC, start=True, stop=True)
                    nc.scalar.activation(cdfp[:, t, :], cps[:, 0:SW], AF.Identity)
                # normalize by cdf[last]: extract via 1-col matmul to partition 0
                lastp = ps.tile([2, SW], F32, tag="qrow", bufs=2)
                nc.tensor.matmul(lastp[0:1, :], triM[:, GEFF - 1:GEFF], dC,
                                 start=True, stop=True)
                rec = sb.tile([1, SW], F32, tag="rec", bufs=2)
                nc.vector.reciprocal(out=rec[:], in_=lastp[0:1, :])
                r128 = ps.tile([128, 512], F32, tag="ps_m", bufs=2)
                nc.tensor.matmul(r128[:, 0:SW], onesrow[:], rec[:],
                                 start=True, stop=True)
                for t in range(4):
                    nc.vector.tensor_tensor(out=cdfp[:, t, :], in0=cdfp[:, t, :],
                                            in1=r128[:, 0:SW], op=ALU.mult)
                for qi in range(Q):
                    qrow = ps.tile([2, SW], F32, tag="qrow", bufs=2)
                    # fused across the 4 grid tiles: [128, 4*SW] (t=3 rows
                    # 116:128 hold garbage; the per-t matmuls skip them)
                    d1 = sb.tile([128, 4, SW], F32, tag="d1", bufs=2)
                    nc.vector.tensor_scalar(out=d1[:].rearrange("p t s -> p (t s)"),
                                            in0=cdfp[:].rearrange("p t s -> p (t s)"),
                                            scalar1=float(-qs[qi]), scalar2=None,
                                            op0=ALU.add)
                    nc.vector.tensor_scalar(out=d1[:].rearrange("p t s -> p (t s)").bitcast(I32),
                                            in0=d1[:].rearrange("p t s -> p (t s)").bitcast(I32),
                                            scalar1=0x7FFFFFFF, scalar2=None,
                                            op0=ALU.bitwise_and)
                    w = sb.tile([128, 4, SW], F32, tag="wt", bufs=2)
                    nc.scalar.activation(w[:].rearrange("p t s -> p (t s)"),
                                         d1[:].rearrange("p t s -> p (t s)"),
                                         AF.Sigmoid, scale=-100.0)
                    for t in range(4):
                        tw = tws[t]
                        nc.tensor.matmul(qrow[:], oiT[0:tw, :, t], w[0:tw, t, :],
                                         start=(t == 0), stop=(t == 3),
                                         skip_group_check=True)
                    st2 = sb.tile([2, SW], F32, tag="st2", bufs=2)
                    nc.scalar.activation(st2[:], qrow[:], AF.Identity)
                    nc.sync.dma_start(out=s0t1_out[2 * qi:2 * qi + 2, s0:s1], in_=st2[:])

            # ---------------- main schedule ----------------
            phases = os.environ.get("KPHASES", "all")
            if phases != "all":
                nc.gpsimd.memset(densC[:], 0.0)
                nc.gpsimd.memset(curT[1][:], 0.0)
                nc.gpsimd.memset(curT[2][:], 0.0)

            def layer(l, with_gat):
                stats_l, xsall = stats_phase(l)
                gen = None
                if with_gat:
                    gat_prologue(l)
                    gen = gat_edges(l, chunk_tile)
                nbatch_total = (C + BATCH - 1) // BATCH
                emitted = 0
                # interleave: preps up front (4 independent chains), then
                # 4x16 slots; emit edge batches between slots so GAT Pool/DMA
                # work overlaps readout compute
                points = 4 + 4 * 16
                per_point = nbatch_total / points if with_gat else 0.0
                acc = 0.0

                def drain():
                    nonlocal emitted, acc
                    acc += per_point
                    while gen is not None and emitted < min(nbatch_total, int(round(acc))):
                        try:
                            next(gen); emitted += 1
                        except StopIteration:
                            return

                preps = []
                for g in range(G):
                    cstack_g, xh_s_g, xsq_g = derive_graph(l, g, xsall, *stats_l[g])
                    preps.append(unit_prep(l, g, cstack_g, xh_s_g, xsq_g))
                    drain()
                for g in range(G):
                    u = l * G + g
                    Lq, X4 = preps[g]
                    for k in range(DPC):
                        slot(u, k, Lq, X4)
                        drain()
                if with_gat:
                    for _ in gen:
                        pass

            for _ in range(reps):
                if phases == "r0":
                    layer(0, False)
                    quantiles(0, 64)
                elif phases == "r0g0":
                    layer(0, True)
                    quantiles(0, 64)
                else:
                    layer(0, True)
                    quantiles(0, 64)
                    layer(1, True)
                    quantiles(64, 128)
                    layer(2, False)
                    quantiles(128, 192)

            if KDBG:
                nc.sync.dma_start(out=dbg_dens[:], in_=densC[:])
            nc.sync.dma_start(out=pmean_out[:], in_=pmean[:])
            nc.sync.dma_start(out=pmax_out[:], in_=pmax[:])
    nc.compile()
    return nc


_CACHE = {}


def _get_program(C, chunk_tile, reps=1):
    key = (C, tuple(chunk_tile.tolist()), reps,
           os.environ.get("KPHASES", "all"), os.environ.get("KDEBUG"))
    if key not in _CACHE:
        _CACHE[key] = build_program(C, chunk_tile, reps)
    return _CACHE[key]


def _host_inputs(inputs, s_idx, d_idx, dlf, maskneg, C):
    x = np.asarray(inputs["x"], np.float32)
    repq = np.zeros((DPC, 3, 512), np.float32)
    repx = np.zeros((DPC, 4, 3, 128), np.float32)
    for k in range(DPC):
        m, s = divmod(k, 4)
        for r in range(3):
            repq[k, r, 128 * m + 32 * s + r] = 1.0
            repx[k, m, r, 32 * s + r] = -0.5 if r == 2 else 1.0
    import ml_dtypes
    kk = np.arange(GCP, dtype=np.float64) - (GC - 1) / 2.0

    def wrap16(idx):
        # idx [128, C] int32 -> [128, CW] i16: global edge j=c*128+e at [j%16, j//16],
        # replicated across the 8 Q7 cores (partition blocks of 16)
        Cn = idx.shape[1]
        CW = ((Cn + BATCH - 1) // BATCH) * BATCH * 8
        flat = idx.T.ravel()                       # j = c*128+e order
        t = np.zeros((16, CW), np.int16)
        jj = np.arange(Cn * 128)
        t[jj % 16, jj // 16] = flat.astype(np.int16)
        return np.tile(t, (8, 1))

    # fused interp+cumsum matrix: cdf500 = densC^T @ triM
    M = np.zeros((GC, GRID))
    pos = np.arange(GRID) * (GC - 1) / (GRID - 1)
    lo = np.floor(pos).astype(int)
    wf = pos - lo
    hi = np.minimum(lo + 1, GC - 1)
    np.add.at(M, (lo, np.arange(GRID)), 1 - wf)
    np.add.at(M, (hi, np.arange(GRID)), wf)
    triM = np.zeros((GC, 512), np.float32)
    triM[:, 0:GRID] = np.cumsum(M, axis=1)

    oiT = np.zeros((128, 2, 4), np.float32)
    oiT[:, 0, :] = 1.0
    oiT[:, 1, :] = (np.arange(128, dtype=np.float32)[:, None]
                    + 128.0 * np.arange(4, dtype=np.float32)[None, :])

    qsv = np.linspace(0.0, 1.0, Q)
    qbias = np.zeros((128, 2 * Q + 1), np.float32)
    qbias[:, 0:2 * Q:2] = 100.0 * qsv[None, :]
    qbias[:, 1:2 * Q:2] = -100.0 * qsv[None, :]
    qbias[:, 2 * Q] = 1e-8

    im_base = dict(
        repq=repq, repx=repx,
        xT=np.ascontiguousarray(x.T),
        sidx=wrap16(s_idx), didx=wrap16(d_idx),
        dlfb=dlf.astype(ml_dtypes.bfloat16),
        mneg=maskneg,
        iota3=np.stack([np.ones(GCP), kk, kk ** 2]).astype(np.float32),
        triM=triM,
        oiT=oiT,
        qbias=qbias,
        iotaF=np.tile(np.arange(128, dtype=np.float32)[None, :], (128, 1)).astype(ml_dtypes.bfloat16),
    )
    for l in range(2):
        A = np.zeros((128, 8), np.float32)
        as_l = np.asarray(inputs[f"as{l}"], np.float32)
        ad_l = np.asarray(inputs[f"ad{l}"], np.float32)
        for h in range(HEADS):
            A[h * HID:(h + 1) * HID, h] = as_l[h]
            A[h * HID:(h + 1) * HID, 4 + h] = ad_l[h]
        im_base[f"W{l}"] = np.asarray(inputs[f"W{l}"], np.float32)
        im_base[f"Av{l}"] = A
        im_base[f"bcol{l}"] = np.asarray(inputs[f"b{l}"], np.float32).reshape(128, 1)
    in_maps = []
    for c in range(N_CORES):
        selm = np.zeros((128, DPC), np.float32)
        for k in range(DPC):
            selm[DPC * c + k, k] = 1.0
        in_maps.append({**im_base, "sel": selm})
    return in_maps


def kernel(**inputs) -> np.ndarray:
    from concourse.bass_utils import run_bass_kernel_spmd
    s_idx, d_idx, dlf, maskneg, chunk_tile = _edge_prep(np.asarray(inputs["edge_index"]))
    C = s_idx.shape[1]
    nc = _get_program(C, chunk_tile)
    in_maps = _host_inputs(inputs, s_idx, d_idx, dlf, maskneg, C)
    res = run_bass_kernel_spmd(nc, in_maps, list(range(N_CORES))).results
    return _assemble(inputs, res)


def _assemble(inputs, res):
    # kf[l, g, d, q]
    kf = np.zeros((3, G, 128, Q), np.float64)
    for c in range(N_CORES):
        s0t1 = np.asarray(res[c]["s0t1"], np.float64)    # [2Q, 192]
        S0, T1 = s0t1[0::2, :], s0t1[1::2, :]            # [Q, 192]
        mndl2 = np.asarray(res[c]["mndl2"], np.float64)  # [16, 24]
        mn = mndl2[:, 0::2].T.reshape(-1)                # [192] slot-ordered
        dl = mndl2[:, 1::2].T.reshape(-1)
        qvT = (mn[None, :] * S0 + dl[None, :] * T1) / (S0 + 1e-8)
        for u in range(UNITS):
            l, g = divmod(u, G)
            kf[l, g, DPC * c:DPC * (c + 1), :] = qvT[:, u * DPC:(u + 1) * DPC].T
    pmean = res[0]["pmean"] / 512.0          # [128, 12]
    pmax = res[0]["pmax"]
    pool_w = np.asarray(inputs["pool_w"], np.float64)
    beta = np.asarray(inputs["beta"], np.float64)
    h0 = float(np.asarray(inputs["h0"]).reshape(-1)[0])
    h_list, k_list = [], []
    for l in range(3):
        wp = (pool_w[0] * pmean[:, l * G:(l + 1) * G] + pool_w[1] * pmax[:, l * G:(l + 1) * G]).T
        lpW = np.asarray(inputs[f"lpW{l}"], np.float64)
        lpb = np.asarray(inputs[f"lpb{l}"], np.float64)
        h_list.append(wp @ lpW + lpb)
        kW = np.asarray(inputs[f"kW{l}"], np.float64)
        kb = np.asarray(inputs[f"kb{l}"], np.float64)
        k_list.append(kf[l].reshape(G, 128 * Q) @ kW + kb)
    main_out = np.mean(h_list, axis=0)
    kde_out = np.mean(k_list, axis=0)
    risk = (main_out + kde_out) @ beta + h0
    return risk.astype(np.float32)


# revision 54
# speedup vs baseline: 1.4038x; 1.2415x over previous
"""Trainium2 Bass kernel for nn_GAT_KDE_14766097563859.

2-layer GAT over a 2048-node graph + per-(graph,layer) KDE soft-quantile
readouts. SPMD over 8 NeuronCores: GAT replicated, KDE sharded by feature dim
(each core owns 16 of 128 dims for all 12 (layer,graph) units = 192 slots).

v3 (over the v2 baseline):
- density evaluated on a 125-point coarse grid; cumsum+linear-interp back to
  the 500-point cdf is folded into one constant [125,500] matmul matrix
  (linear maps compose), cutting exp/matmul volume 4x
- per-slot node-sum fused into the exp activation via accum_out (zero DVE)
- quantile stage flipped to [fine-grid part, slot free] orientation:
  sigma(-100|d|) = min(sigma(-100d), sigma(100d)) with the q-shift folded
  into the sigmoid bias; S0/T1 reductions over the grid run on PE
- PSUM->SBUF copies moved to Act; xh_s/xsq on Act (scale-AP / Square, with
  the -0.5 folded into the repx constant); per-graph xs kept, not re-derived
"""
import os
import sys
sys.path.insert(0, "/opt/trn_rl_repo")
import numpy as np

import concourse.bass as bass
import concourse.bacc as bacc
import concourse.mybir as mybir
from concourse.tile import TileContext
from concourse.masks import make_identity
from concourse import library_config

F32 = mybir.dt.float32
F32R = mybir.dt.float32r
BF16 = mybir.dt.bfloat16
I32 = mybir.dt.int32
I16 = mybir.dt.int16
AF = mybir.ActivationFunctionType
ALU = mybir.AluOpType
AX = mybir.AxisListType

G, NG, N, E = 4, 512, 2048, 32768
IN_DIM, HID, HEADS, HC, OUT_DIM = 128, 32, 4, 128, 32
N_LAYERS, GRID, Q, NEG_SLOPE = 2, 500, 20, 0.2
N_CORES = 8
UNITS = (N_LAYERS + 1) * G            # 12, unit u = l*G + g
DPC = 16                              # dims per core
SLOTS = UNITS * DPC                   # 192
GEFF = 500                            # fine grid (matches reference GRID)
GC = 125                              # coarse density grid
GCP = 128                             # GC padded (fp32r matmul needs free%4==0)
BATCH = 8                             # chunks per gather batch (1024 idxs = SWDGE ring cap)
TBLW = 256                            # bf16 cols of xh_al row (512B, dma_gather needs %256B)
ALDW = 128                            # bf16 cols of al_d row (256B)


def _edge_prep(edge_index):
    src = edge_index[0].astype(np.int64)
    dst = edge_index[1].astype(np.int64)
    s_all = np.concatenate([src, np.arange(N)])
    d_all = np.concatenate([dst, np.arange(N)])
    order = np.argsort(d_all, kind="stable")
    s_s, d_s = s_all[order], d_all[order]
    cs, cd, ct, cm = [], [], [], []
    for t in range(16):
        sel = (d_s // 128) == t
        se, de = s_s[sel], d_s[sel]
        ne = len(se)
        npad = (-ne) % 128
        se = np.concatenate([se, np.zeros(npad, np.int64)])
        de = np.concatenate([de, np.full(npad, t * 128, np.int64)])
        rm = np.concatenate([np.ones(ne, bool), np.zeros(npad, bool)])
        for c0 in range(0, len(se), 128):
            cs.append(se[c0:c0 + 128]); cd.append(de[c0:c0 + 128])
            cm.append(rm[c0:c0 + 128]); ct.append(t)
    C = len(cs)
    s_idx = np.stack(cs, 1).astype(np.int32)                  # [128, C] src ids
    d_idx = np.stack(cd, 1).astype(np.int32)                  # [128, C] dst ids
    dlf = (np.stack(cd, 1) - np.asarray(ct)[None, :] * 128).astype(np.float32)
    maskneg = np.stack(cm, 1).astype(np.float32)  # 1 valid, 0 pad
    return s_idx, d_idx, dlf, maskneg, np.asarray(ct, np.int32)


def build_program(C, chunk_tile, reps=1):
    nc = bacc.Bacc(None, target_bir_lowering=False, debug=True)

    xT_in = nc.declare_dram_parameter("xT", [128, N], F32, isOutput=False)
    Wi = [nc.declare_dram_parameter(f"W{l}", [128, HC], F32, isOutput=False) for l in range(2)]
    Avi = [nc.declare_dram_parameter(f"Av{l}", [128, 8], F32, isOutput=False) for l in range(2)]
    bci = [nc.declare_dram_parameter(f"bcol{l}", [128, 1], F32, isOutput=False) for l in range(2)]
    sel_in = nc.declare_dram_parameter("sel", [128, DPC], F32, isOutput=False)
    iota3_in = nc.declare_dram_parameter("iota3", [3, GCP], F32R, isOutput=False)
    triM_in = nc.declare_dram_parameter("triM", [GC, 512], F32, isOutput=False)
    oiT_in = nc.declare_dram_parameter("oiT", [128, 2, 4], F32, isOutput=False)
    qbias_in = nc.declare_dram_parameter("qbias", [128, 2 * Q + 1], F32, isOutput=False)
    iotaF_in = nc.declare_dram_parameter("iotaF", [128, 128], BF16, isOutput=False)
    CW = ((C + BATCH - 1) // BATCH) * BATCH * 8
    sidx_in = nc.declare_dram_parameter("sidx", [128, CW], I16, isOutput=False)
    didx_in = nc.declare_dram_parameter("didx", [128, CW], I16, isOutput=False)
    dlfb_in = nc.declare_dram_parameter("dlfb", [128, C], BF16, isOutput=False)
    mneg_in = nc.declare_dram_parameter("mneg", [128, C], F32, isOutput=False)
    repq_in = nc.declare_dram_parameter("repq", [DPC, 3, 512], F32R, isOutput=False)
    repx_in = nc.declare_dram_parameter("repx", [DPC, 4, 3, 128], F32R, isOutput=False)
    ohtbl_in = nc.declare_dram_parameter("ohtbl", [128, C * 128], BF16, isOutput=False)

    s0t1_out = nc.declare_dram_parameter("s0t1", [2 * Q, SLOTS], F32, isOutput=True)
    mndl_out = nc.declare_dram_parameter("mndl2", [DPC, 2 * UNITS], F32, isOutput=True)
    pmean_out = nc.declare_dram_parameter("pmean", [128, UNITS], F32, isOutput=True)
    pmax_out = nc.declare_dram_parameter("pmax", [128, UNITS], F32, isOutput=True)
    KDBG = os.environ.get("KDEBUG") == "1"
    if KDBG:
        dbg_dens = nc.declare_dram_parameter("dbg_dens", [GC, SLOTS], F32, isOutput=True)

    xh_al_hbm = nc.dram_tensor("xh_al", [N, TBLW], BF16)
    ald_hbm = nc.dram_tensor("ald", [N, ALDW], BF16)

    with TileContext(nc) as tc:
        with (
            tc.tile_pool(name="cst", bufs=1) as cst,
            tc.tile_pool(name="wrk", bufs=2) as sb,
            tc.tile_pool(name="ps", bufs=1, space="PSUM") as ps,
        ):
            # ---------------- resident state ----------------
            curT = [cst.tile([128, N], F32, tag=f"curT{l}", name=f"curT{l}") for l in range(3)]
            W = [cst.tile([128, HC], F32, tag=f"Wt{l}", name=f"Wt{l}") for l in range(2)]
            Av = [cst.tile([128, 8], F32, tag=f"Avt{l}", name=f"Avt{l}") for l in range(2)]
            bcol = [cst.tile([128, 1], F32, tag=f"bct{l}", name=f"bct{l}") for l in range(2)]
            sel = cst.tile([128, DPC], F32)
            iota3 = cst.tile([3, GCP], F32R)
            triM = cst.tile([GC, 512], F32)
            oiT = cst.tile([128, 2, 4], F32)
            qbias = cst.tile([128, 2 * Q + 1], F32)
            iotaF = cst.tile([128, 128], BF16)
            sidx = cst.tile([128, CW], I16)
            didx = cst.tile([128, CW], I16)
            dlfb = cst.tile([128, C], BF16)
            mneg = cst.tile([128, C], F32)
            repq = cst.tile([DPC, 3, 512], F32R)
            repx = cst.tile([DPC, 4, 3, 128], F32R)
            ones16 = cst.tile([DPC, 512], F32R)
            onesrow = cst.tile([1, 128], F32)
            densC = cst.tile([GC, SLOTS], F32)
            pmean = cst.tile([128, UNITS], F32)
            pmax = cst.tile([128, UNITS], F32)
            id128 = cst.tile([128, 128], F32)
            xhT = cst.tile([128, N], F32)
            al = cst.tile([128, 16, 8], F32)
            stage = cst.tile([128, 16, TBLW], BF16)
            stage_d = cst.tile([128, 16, ALDW], BF16)
            nc.gpsimd.load_library(library_config.mlp)
            make_identity(nc, id128[:])
            nc.gpsimd.dma_start(out=curT[0][:], in_=xT_in[:])
            for l in range(2):
                nc.gpsimd.dma_start(out=W[l][:], in_=Wi[l][:])
                nc.gpsimd.dma_start(out=Av[l][:], in_=Avi[l][:])
                nc.gpsimd.dma_start(out=bcol[l][:], in_=bci[l][:])
            nc.gpsimd.dma_start(out=sel[:], in_=sel_in[:])
            nc.gpsimd.dma_start(out=iota3[:], in_=iota3_in[:])
            nc.gpsimd.dma_start(out=triM[:], in_=triM_in[:])
            nc.gpsimd.dma_start(out=oiT[:], in_=oiT_in[:])
            nc.gpsimd.dma_start(out=qbias[:], in_=qbias_in[:])
            nc.gpsimd.dma_start(out=iotaF[:], in_=iotaF_in[:])
            nc.gpsimd.dma_start(out=sidx[:], in_=sidx_in[:])
            nc.gpsimd.dma_start(out=didx[:], in_=didx_in[:])
            nc.gpsimd.dma_start(out=dlfb[:], in_=dlfb_in[:])
            nc.gpsimd.dma_start(out=mneg[:], in_=mneg_in[:])
            nc.gpsimd.dma_start(out=repq[:], in_=repq_in[:])
            nc.gpsimd.dma_start(out=repx[:], in_=repx_in[:])
            nc.gpsimd.memset(ones16[:].bitcast(F32), 1.0)
            nc.gpsimd.memset(onesrow[:], 1.0)
            nc.gpsimd.memset(stage[:].bitcast(mybir.dt.uint16), 0)
            nc.gpsimd.memset(stage_d[:].bitcast(mybir.dt.uint16), 0)
            # constant 1.0 columns of the gather table (col 33h+32), set once
            nc.gpsimd.memset(stage[:, :, 0:132].rearrange("p t (h c) -> p t h c", c=33)[:, :, :, 32:33].bitcast(mybir.dt.uint16), 0x3F80)

            # ---------------- per-layer stats (all 4 graphs) ----------------
            def stats_phase(l):
                cur = curT[l]
                xsall = sb.tile([DPC, G, 512], F32, tag="xsall", bufs=2)
                per_g = []
                for g in range(G):
                    u = l * G + g
                    blk = cur[:, 512 * g:512 * (g + 1)]
                    nc.vector.tensor_reduce(out=pmean[:, u:u + 1], in_=blk, axis=AX.X, op=ALU.add)
                    nc.vector.tensor_reduce(out=pmax[:, u:u + 1], in_=blk, axis=AX.X, op=ALU.max)
                    ps_xs = ps.tile([DPC, 512], F32, tag="ps_m", bufs=2)
                    nc.tensor.matmul(ps_xs[:], sel[:], blk, start=True, stop=True)
                    xs = xsall[:, g, :]
                    nc.scalar.activation(xs, ps_xs[:], AF.Identity)
                    mn = sb.tile([DPC, 1], F32, tag=f"mn{g}", name=f"mn{g}", bufs=2)
                    mx = sb.tile([DPC, 1], F32, tag="mx")
                    sm = sb.tile([DPC, 1], F32, tag="sm")
                    sq = sb.tile([DPC, 1], F32, tag="sq")
                    nc.vector.tensor_reduce(out=mn[:], in_=xs, axis=AX.X, op=ALU.min)
                    nc.vector.tensor_reduce(out=mx[:], in_=xs, axis=AX.X, op=ALU.max)
                    nc.vector.tensor_reduce(out=sm[:], in_=xs, axis=AX.X, op=ALU.add)
                    xsq0 = sb.tile([DPC, 512], F32, tag="xsq0")
                    nc.vector.tensor_tensor(out=xsq0[:], in0=xs, in1=xs, op=ALU.mult)
                    nc.vector.tensor_reduce(out=sq[:], in_=xsq0[:], axis=AX.X, op=ALU.add)
                    var = sb.tile([DPC, 1], F32, tag=f"var{g}", name=f"var{g}", bufs=2)
                    mean = sb.tile([DPC, 1], F32, tag="mean")
                    nc.vector.tensor_scalar(out=mean[:], in0=sm[:], scalar1=1.0 / 512,
                                            scalar2=None, op0=ALU.mult)
                    nc.vector.tensor_scalar(out=var[:], in0=sq[:], scalar1=1.0 / 512,
                                            scalar2=None, op0=ALU.mult)
                    m2 = sb.tile([DPC, 1], F32, tag="m2")
                    nc.vector.tensor_tensor(out=m2[:], in0=mean[:], in1=mean[:], op=ALU.mult)
                    nc.vector.tensor_tensor(out=var[:], in0=var[:], in1=m2[:], op=ALU.subtract)
                    nc.vector.tensor_scalar(out=mn[:], in0=mn[:], scalar1=-1e-6, scalar2=None, op0=ALU.add)
                    nc.vector.tensor_scalar(out=mx[:], in0=mx[:], scalar1=1e-6, scalar2=None, op0=ALU.add)
                    dl = sb.tile([DPC, 1], F32, tag=f"dl{g}", name=f"dl{g}", bufs=2)
                    nc.vector.tensor_tensor(out=dl[:], in0=mx[:], in1=mn[:], op=ALU.subtract)
                    nc.vector.tensor_scalar(out=dl[:], in0=dl[:], scalar1=1.0 / (GEFF - 1), scalar2=None, op0=ALU.mult)
                    per_g.append((mn, dl, var))
                # std = sqrt(var) on DVE (rsqrt bit-trick + 3 Newton steps)
                # so Act never loads the sqrt table (avoids table thrash
                # against the overlapped quantile-pass sigmoids)
                out = []
                for g, (mn, dl, var) in enumerate(per_g):
                    u = l * G + g
                    y = sb.tile([DPC, 1], F32, tag="rsq_y", bufs=2)
                    nc.vector.tensor_scalar(out=y[:].bitcast(I32), in0=var[:].bitcast(I32),
                                            scalar1=1, scalar2=None,
                                            op0=ALU.logical_shift_right)
                    nc.vector.tensor_scalar(out=y[:].bitcast(I32), in0=y[:].bitcast(I32),
                                            scalar1=-1, scalar2=None,
                                            op0=ALU.bitwise_xor)
                    nc.vector.tensor_scalar(out=y[:].bitcast(I32), in0=y[:].bitcast(I32),
                                            scalar1=0x5F3759E0, scalar2=None,
                                            op0=ALU.add)
                    vh = sb.tile([DPC, 1], F32, tag="rsq_vh", bufs=2)
                    nc.vector.tensor_scalar(out=vh[:], in0=var[:], scalar1=-0.5,
                                            scalar2=None, op0=ALU.mult)
                    for _ in range(3):
                        y2 = sb.tile([DPC, 1], F32, tag="rsq_y2", bufs=2)
                        nc.vector.tensor_tensor(out=y2[:], in0=y[:], in1=y[:], op=ALU.mult)
                        nc.vector.scalar_tensor_tensor(out=y2[:], in0=y2[:], scalar=1.0,
                                                       in1=vh[:], op0=ALU.mult,
                                                       op1=ALU.mult)
                        nc.vector.tensor_scalar(out=y2[:], in0=y2[:], scalar1=1.5,
                                                scalar2=None, op0=ALU.add)
                        nc.vector.tensor_tensor(out=y[:], in0=y[:], in1=y2[:], op=ALU.mult)
                    std = sb.tile([DPC, 1], F32, tag=f"std{g}", name=f"std{g}", bufs=2)
                    nc.vector.tensor_tensor(out=std[:], in0=var[:], in1=y[:], op=ALU.mult)
                    nc.sync.dma_start(out=mndl_out[:, 2 * u:2 * u + 1], in_=mn[:])
                    nc.sync.dma_start(out=mndl_out[:, 2 * u + 1:2 * u + 2], in_=dl[:])
                    out.append((mn, dl, std))
                return out, xsall

            def derive_graph(l, g, xsall, mn, dl, std):
                """Per-graph slot-pass prep: build cstack + X rows from kept xs."""
                xs = xsall[:, g, :]
                h = sb.tile([DPC, 1], F32, tag="h")
                nc.vector.tensor_scalar(out=h[:], in0=std[:], scalar1=float(1e-8 / 3),
                                        scalar2=float(1.06 * 512 ** -0.2),
                                        op0=ALU.add, op1=ALU.mult)
                rh = sb.tile([DPC, 1], F32, tag="rh", bufs=2)
                nc.vector.reciprocal(out=rh[:], in_=h[:])
                # recentered basis: k' = k-(GC-1)/2, x' = (x-c)/h with c the
                # grid midpoint -> a0 = 0 and all poly terms stay O(span/2h),
                # minimizing f32r (tf32) rounding of the stored poly values
                a0 = sb.tile([DPC, 1], F32, tag="a0")
                a1 = sb.tile([DPC, 1], F32, tag="a1")
                nc.gpsimd.memset(a0[:], 0.0)
                # coarse grid step is (GEFF-1)/(GC-1) fine steps
                nc.vector.scalar_tensor_tensor(
                    out=a1[:], in0=dl[:], scalar=float((GEFF - 1) / (GC - 1)),
                    in1=rh[:], op0=ALU.mult, op1=ALU.mult)
                mnrh = sb.tile([DPC, 1], F32, tag="mnrh")
                nc.vector.tensor_tensor(out=mnrh[:], in0=mn[:], in1=rh[:], op=ALU.mult)
                biasc = sb.tile([DPC, 1], F32, tag="biasc", bufs=2)
                nc.vector.scalar_tensor_tensor(
                    out=biasc[:], in0=a1[:], scalar=-float((GC - 1) / 2),
                    in1=mnrh[:], op0=ALU.mult, op1=ALU.subtract)
                # cstack [16, 9]: r0=(q0,q1,q2) r1=(a0,a1,0) r2=(1,0,0)
                cstack = sb.tile([DPC, 9], F32R, tag="cs", bufs=2)
                nc.gpsimd.memset(cstack[:].bitcast(F32), 0.0)
                nc.vector.tensor_tensor(out=cstack[:, 0:1], in0=a0[:], in1=a0[:], op=ALU.mult)
                nc.vector.tensor_scalar(out=cstack[:, 0:1], in0=cstack[:, 0:1], scalar1=-0.5, scalar2=None, op0=ALU.mult)
                nc.vector.tensor_tensor(out=cstack[:, 1:2], in0=a0[:], in1=a1[:], op=ALU.mult)
                nc.vector.tensor_scalar(out=cstack[:, 1:2], in0=cstack[:, 1:2], scalar1=-1.0, scalar2=None, op0=ALU.mult)
                nc.vector.tensor_tensor(out=cstack[:, 2:3], in0=a1[:], in1=a1[:], op=ALU.mult)
                nc.vector.tensor_scalar(out=cstack[:, 2:3], in0=cstack[:, 2:3], scalar1=-0.5, scalar2=None, op0=ALU.mult)
                nc.vector.tensor_copy(cstack[:, 3:4], a0[:])
                nc.vector.tensor_copy(cstack[:, 4:5], a1[:])
                nc.gpsimd.memset(cstack[:, 6:7].bitcast(F32), 1.0)
                xh_s = sb.tile([DPC, 512], F32R, tag="xh_s", bufs=2)
                nc.scalar.activation(xh_s[:], xs, AF.Identity, scale=rh[:, 0:1],
                                     bias=biasc[:, 0:1])
                xsq = sb.tile([DPC, 512], F32R, tag="xsq", bufs=2)
                nc.scalar.activation(xsq[:], xh_s[:], AF.Square)
                return cstack, xh_s, xsq

            # ---------------- per-(unit) prep: Lq / X4 ----------------
            def unit_prep(l, g, cstack_g, xh_s_g, xsq_g):
                # lh [3, 512]: col 128m+32s+r' = cstack triple r' of slot k=4m+s
                ps_lh = ps.tile([3, 512], F32, tag="ps_m", bufs=2)
                for r in range(3):
                    nc.tensor.matmul(ps_lh[:], cstack_g[:, 3 * r:3 * r + 3],
                                     repq[:, r, :],
                                     start=(r == 0), stop=(r == 2),
                                     skip_group_check=True)
                lh = sb.tile([3, 512], F32R, tag="lh", bufs=2)
                nc.scalar.activation(lh[:], ps_lh[:], AF.Identity)
                Lq = []
                X4 = []
                for m in range(4):
                    ps_L = ps.tile([128, GCP], F32, tag="ps_m", bufs=2)
                    nc.tensor.matmul(ps_L[:], lh[:, 128 * m:128 * (m + 1)], iota3[:],
                                     start=True, stop=True)
                    Lqm = sb.tile([128, GCP], F32R, tag=f"Lq{m}", name=f"Lqm{m}", bufs=4)
                    nc.scalar.activation(Lqm[:], ps_L[:], AF.Identity)
                    Lq.append(Lqm)
                    ps_X = ps.tile([128, 512], F32, tag="ps_m", bufs=2)
                    nc.tensor.matmul(ps_X[:], repx[:, m, 0, :],
                                     ones16[:], start=True, stop=False, skip_group_check=True)
                    nc.tensor.matmul(ps_X[:], repx[:, m, 1, :], xh_s_g,
                                     start=False, stop=False, skip_group_check=True)
                    # repx r=2 entries are -0.5: folds the -u^2/2 scaling
                    nc.tensor.matmul(ps_X[:], repx[:, m, 2, :], xsq_g,
                                     start=False, stop=True, skip_group_check=True)
                    X4m = sb.tile([128, 512], F32R, tag=f"X4{m}", name=f"X4m{m}", bufs=4)
                    nc.scalar.activation(X4m[:], ps_X[:], AF.Identity)
                    X4.append(X4m)
                return Lq, X4

            # ---------------- one KDE slot ----------------
            def slot(u, k, Lq, X4):
                m, s = divmod(k, 4)
                sidx_ = u * DPC + k
                psu = ps.tile([GC, 512], F32, tag="psu", bufs=2)
                nc.tensor.matmul(psu[:], Lq[m][32 * s:32 * s + 3, 0:GC],
                                 X4[m][32 * s:32 * s + 3, :], start=True, stop=True,
                                 tile_position=(32 * s, 0), skip_group_check=True)
                dump = sb.tile([GC, 512], BF16, tag="dump", bufs=2)
                nc.scalar.activation(dump[:], psu[:], AF.Exp,
                                     accum_out=densC[:, sidx_:sidx_ + 1])

            # ---------------- GAT prologue ----------------
            def gat_prologue(l):
                cur = curT[l]
                for b in range(4):
                    pxh = ps.tile([128, 512], F32, tag="ps_m", bufs=2)
                    nc.tensor.matmul(pxh[:], W[l][:], cur[:, 512 * b:512 * (b + 1)],
                                     start=True, stop=True)
                    nc.vector.tensor_copy(xhT[:, 512 * b:512 * (b + 1)], pxh[:])
                for t in range(16):
                    pal = ps.tile([128, 8], F32, tag="ps_m", bufs=2)
                    nc.tensor.matmul(pal[:], xhT[:, 128 * t:128 * (t + 1)], Av[l][:],
                                     start=True, stop=True)
                    nc.vector.tensor_copy(al[:, t, :], pal[:])
                    pxr = ps.tile([128, 128], F32, tag="ps_m", bufs=2)
                    nc.tensor.transpose(pxr[:], xhT[:, 128 * t:128 * (t + 1)], id128[:])
                    nc.vector.tensor_copy(
                        stage[:, t, 0:132].rearrange("p (h c) -> p h c", h=4)[:, :, 0:32],
                        pxr[:].rearrange("p (h c) -> p h c", h=4))
                # al_s -> bf16 table cols 132:140 viewed as f32 x4
                nc.vector.tensor_copy(stage[:, :, 132:140].bitcast(F32), al[:, :, 0:4])
                nc.vector.tensor_copy(stage_d[:, :, 0:8].bitcast(F32), al[:, :, 4:8])
                nc.sync.dma_start(out=xh_al_hbm[:].rearrange("(t p) d -> p t d", p=128),
                                  in_=stage[:])
                nc.sync.dma_start(out=ald_hbm[:].rearrange("(t p) d -> p t d", p=128),
                                  in_=stage_d[:])

            # ---------------- GAT edge batches ----------------
            def finish_tile(l, t, raw):
                rawv = raw[:].rearrange("p (h c) -> p h c", h=4)
                rd = sb.tile([128, 4], F32, tag="rd")
                nc.vector.tensor_scalar(out=rd[:], in0=rawv[:, :, 32], scalar1=1e-16,
                                        scalar2=None, op0=ALU.add)
                nc.vector.reciprocal(out=rd[:], in_=rd[:])
                o = sb.tile([128, HC], F32, tag="otile", bufs=2)
                nc.vector.tensor_tensor(
                    out=o[:].rearrange("p (h c2) -> p h c2", h=4),
                    in0=rawv[:, :, 0:32],
                    in1=rd[:].rearrange("p h -> p h ()").to_broadcast([128, 4, 32]),
                    op=ALU.mult)
                pt = ps.tile([128, 128], F32, tag="ps_m", bufs=2)
                nc.tensor.transpose(pt[:], o[:], id128[:])
                nc.scalar.activation(curT[l + 1][:, 128 * t:128 * (t + 1)], pt[:],
                                     AF.Relu if l == 0 else AF.Identity,
                                     bias=bcol[l][:, 0:1])

            def gat_edges(l, chunk_tile):
                """Generator: yields after each emitted batch."""
                raw = None
                cur_t = -1
                for b0 in range(0, C, BATCH):
                    cn = min(BATCH, C - b0)
                    gwin = sb.tile([128, BATCH, TBLW], BF16, tag="gwin", bufs=2)
                    nc.gpsimd.dma_gather(gwin[:], xh_al_hbm[:],
                                         sidx[:, b0 * 8:(b0 + BATCH) * 8],
                                         BATCH * 128, BATCH * 128, TBLW, queue_num=0)
                    aldw = sb.tile([128, BATCH, ALDW], BF16, tag="aldw", bufs=2)
                    nc.gpsimd.dma_gather(aldw[:], ald_hbm[:],
                                         didx[:, b0 * 8:(b0 + BATCH) * 8],
                                         BATCH * 128, BATCH * 128, ALDW, queue_num=0)
                    # z = al_s[src] + al_d[dst]; leaky-relu; +maskneg; exp -> bf16
                    z = sb.tile([128, BATCH, 4], F32, tag="z", bufs=2)
                    nc.vector.tensor_tensor(out=z[:, 0:cn, :],
                                            in0=gwin[:, 0:cn, 132:140].bitcast(F32),
                                            in1=aldw[:, 0:cn, 0:8].bitcast(F32), op=ALU.add)
                    # leaky-relu fused: max(z, 0.2*z)
                    zl = sb.tile([128, BATCH, 4], F32, tag="zl", bufs=2)
                    nc.vector.scalar_tensor_tensor(
                        out=zl[:, 0:cn, :], in0=z[:, 0:cn, :], scalar=NEG_SLOPE,
                        in1=z[:, 0:cn, :], op0=ALU.mult, op1=ALU.max)
                    p_r = sb.tile([128, BATCH, 4], BF16, tag="p_r", bufs=2)
                    nc.scalar.activation(p_r[:, 0:cn, :], zl[:, 0:cn, :], AF.Exp)
                    # one-hot [e, d] streamed from HBM (static; padding rows zero)
                    OH = sb.tile([128, BATCH, 128], BF16, tag="OH", bufs=2)
                    nc.scalar.dma_start(
                        out=OH[:, 0:cn, :],
                        in_=ohtbl_in[:, b0 * 128:(b0 + cn) * 128].rearrange(
                            "p (c f) -> p c f", f=128))
                    # sxh = gathered (feat|1.0) * alpha  (4x33 interleave)
                    sxh = sb.tile([128, BATCH, 132], BF16, tag="sxh", bufs=2)
                    nc.vector.tensor_tensor(
                        out=sxh[:, 0:cn, :].rearrange("p b (h c) -> p b h c", h=4),
                        in0=gwin[:, 0:cn, 0:132].rearrange("p b (h c) -> p b h c", h=4),
                        in1=p_r[:, 0:cn, :].rearrange("p b h -> p b h ()").to_broadcast([128, cn, 4, 33]),
                        op=ALU.mult)
                    for ci in range(cn):
                        c = b0 + ci
                        t = int(chunk_tile[c])
                        first = (c == 0) or (int(chunk_tile[c - 1]) != t)
                        last = (c == C - 1) or (int(chunk_tile[c + 1]) != t)
                        if first:
                            if raw is not None:
                                finish_tile(l, cur_t, raw)
                            raw = ps.tile([128, 132], F32, tag="raw",
                                          padded_shape=[128, 512], bufs=2)
                            cur_t = t
                        nc.tensor.matmul(raw[:], OH[:, ci, :], sxh[:, ci, :],
                                         start=first, stop=last, skip_group_check=True)
                    yield
                if raw is not None:
                    finish_tile(l, cur_t, raw)

            # ---------------- quantile stage (per-layer slice pass) ----------------
            def quantiles(s0, s1):
                qs = np.linspace(0.0, 1.0, Q)
                tws = [min(128, GEFF - 128 * t) for t in range(4)]
                SW = s1 - s0
                dC = densC[:, s0:s1]
                # cdf at fine grid, [fine part (4x128), slot-slice free] via PE
                cdfp = sb.tile([128, 4, SW], F32, tag="cdfp", bufs=2)
                for t in range(4):
                    cps = ps.tile([128, 512], F32, tag="ps_m", bufs=2)
                    nc.tensor.matmul(cps[:, 0:SW], triM[:, 128 * t:128 * (t + 1)],
                                     dC, start=True, stop=True)
                    nc.scalar.activation(cdfp[:, t, :], cps[:, 0:SW], AF.Identity)
                # normalize by cdf[last]: extract via 1-col matmul to partition 0
                lastp = ps.tile([2, SW], F32, tag="qrow", bufs=2)
                nc.tensor.matmul(lastp[0:1, :], triM[:, GEFF - 1:GEFF], dC,
                                 start=True, stop=True)
                rec = sb.tile([1, SW], F32, tag="rec", bufs=2)
                nc.vector.reciprocal(out=rec[:], in_=lastp[0:1, :])
                r128 = ps.tile([128, 512], F32, tag="ps_m", bufs=2)
                nc.tensor.matmul(r128[:, 0:SW], onesrow[:], rec[:],
                                 start=True, stop=True)
                for t in range(4):
                    nc.vector.tensor_tensor(out=cdfp[:, t, :], in0=cdfp[:, t, :],
                                            in1=r128[:, 0:SW], op=ALU.mult)
                for qi in range(Q):
                    qrow = ps.tile([2, SW], F32, tag="qrow", bufs=2)
                    # fused across the 4 grid tiles: [128, 4*SW] (t=3 rows
                    # 116:128 hold garbage; the per-t matmuls skip them)
                    d1 = sb.tile([128, 4, SW], F32, tag="d1", bufs=2)
                    nc.vector.tensor_scalar(out=d1[:].rearrange("p t s -> p (t s)"),
                                            in0=cdfp[:].rearrange("p t s -> p (t s)"),
                                            scalar1=float(-qs[qi]), scalar2=None,
                                            op0=ALU.add)
                    nc.vector.tensor_scalar(out=d1[:].rearrange("p t s -> p (t s)").bitcast(I32),
                                            in0=d1[:].rearrange("p t s -> p (t s)").bitcast(I32),
                                            scalar1=0x7FFFFFFF, scalar2=None,
                                            op0=ALU.bitwise_and)
                    w = sb.tile([128, 4, SW], F32, tag="wt", bufs=2)
                    nc.scalar.activation(w[:].rearrange("p t s -> p (t s)"),
                                         d1[:].rearrange("p t s -> p (t s)"),
                                         AF.Sigmoid, scale=-100.0)
                    for t in range(4):
                        tw = tws[t]
                        nc.tensor.matmul(qrow[:], oiT[0:tw, :, t], w[0:tw, t, :],
                                         start=(t == 0), stop=(t == 3),
                                         skip_group_check=True)
                    st2 = sb.tile([2, SW], F32, tag="st2", bufs=2)
                    nc.scalar.activation(st2[:], qrow[:], AF.Identity)
                    nc.sync.dma_start(out=s0t1_out[2 * qi:2 * qi + 2, s0:s1], in_=st2[:])

            # ---------------- main schedule ----------------
            phases = os.environ.get("KPHASES", "all")
            if phases != "all":
                nc.gpsimd.memset(densC[:], 0.0)
                nc.gpsimd.memset(curT[1][:], 0.0)
                nc.gpsimd.memset(curT[2][:], 0.0)

            def layer(l, with_gat):
                stats_l, xsall = stats_phase(l)
                gen = None
                if with_gat:
                    gat_prologue(l)
                    gen = gat_edges(l, chunk_tile)
                nbatch_total = (C + BATCH - 1) // BATCH
                emitted = 0
                # interleave: preps up front (4 independent chains), then
                # 4x16 slots; emit edge batches between slots so GAT Pool/DMA
                # work overlaps readout compute
                points = 4 + 4 * 16
                per_point = nbatch_total / points if with_gat else 0.0
                acc = 0.0

                def drain():
                    nonlocal emitted, acc
                    acc += per_point
                    while gen is not None and emitted < min(nbatch_total, int(round(acc))):
                        try:
                            next(gen); emitted += 1
                        except StopIteration:
                            return

                preps = []
                for g in range(G):
                    cstack_g, xh_s_g, xsq_g = derive_graph(l, g, xsall, *stats_l[g])
                    preps.append(unit_prep(l, g, cstack_g, xh_s_g, xsq_g))
                    drain()
                for g in range(G):
                    u = l * G + g
                    Lq, X4 = preps[g]
                    for k in range(DPC):
                        slot(u, k, Lq, X4)
                        drain()
                if with_gat:
                    for _ in gen:
                        pass

            for _ in range(reps):
                if phases == "r0":
                    layer(0, False)
                    quantiles(0, 64)
                elif phases == "r0g0":
                    layer(0, True)
                    quantiles(0, 64)
                else:
                    layer(0, True)
                    quantiles(0, 64)
                    layer(1, True)
                    quantiles(64, 128)
                    layer(2, False)
                    quantiles(128, 192)

            if KDBG:
                nc.sync.dma_start(out=dbg_dens[:], in_=densC[:])
            nc.sync.dma_start(out=pmean_out[:], in_=pmean[:])
            nc.sync.dma_start(out=pmax_out[:], in_=pmax[:])
    nc.compile()
    return nc


_CACHE = {}


def _get_program(C, chunk_tile, reps=1):
    key = (C, tuple(chunk_tile.tolist()), reps,
           os.environ.get("KPHASES", "all"), os.environ.get("KDEBUG"))
    if key not in _CACHE:
        _CACHE[key] = build_program(C, chunk_tile, reps)
    return _CACHE[key]


def _host_inputs(inputs, s_idx, d_idx, dlf, maskneg, C):
    x = np.asarray(inputs["x"], np.float32)
    repq = np.zeros((DPC, 3, 512), np.float32)
    repx = np.zeros((DPC, 4, 3, 128), np.float32)
    for k in range(DPC):
        m, s = divmod(k, 4)
        for r in range(3):
            repq[k, r, 128 * m + 32 * s + r] = 1.0
            repx[k, m, r, 32 * s + r] = -0.5 if r == 2 else 1.0
    import ml_dtypes
    kk = np.arange(GCP, dtype=np.float64) - (GC - 1) / 2.0

    def wrap16(idx):
        # idx [128, C] int32 -> [128, CW] i16: global edge j=c*128+e at [j%16, j//16],
        # replicated across the 8 Q7 cores (partition blocks of 16)
        Cn = idx.shape[1]
        CW = ((Cn + BATCH - 1) // BATCH) * BATCH * 8
        flat = idx.T.ravel()                       # j = c*128+e order
        t = np.zeros((16, CW), np.int16)
        jj = np.arange(Cn * 128)
        t[jj % 16, jj // 16] = flat.astype(np.int16)
        return np.tile(t, (8, 1))

    # fused interp+cumsum matrix: cdf500 = densC^T @ triM
    M = np.zeros((GC, GRID))
    pos = np.arange(GRID) * (GC - 1) / (GRID - 1)
    lo = np.floor(pos).astype(int)
    wf = pos - lo
    hi = np.minimum(lo + 1, GC - 1)
    np.add.at(M, (lo, np.arange(GRID)), 1 - wf)
    np.add.at(M, (hi, np.arange(GRID)), wf)
    triM = np.zeros((GC, 512), np.float32)
    triM[:, 0:GRID] = np.cumsum(M, axis=1)

    oiT = np.zeros((128, 2, 4), np.float32)
    oiT[:, 0, :] = 1.0
    oiT[:, 1, :] = (np.arange(128, dtype=np.float32)[:, None]
                    + 128.0 * np.arange(4, dtype=np.float32)[None, :])

    qsv = np.linspace(0.0, 1.0, Q)
    qbias = np.zeros((128, 2 * Q + 1), np.float32)
    qbias[:, 0:2 * Q:2] = 100.0 * qsv[None, :]
    qbias[:, 1:2 * Q:2] = -100.0 * qsv[None, :]
    qbias[:, 2 * Q] = 1e-8

    ohtbl = (dlf[:, :, None] == np.arange(128, dtype=np.float32)[None, None, :])
    ohtbl = (ohtbl * maskneg[:, :, None]).astype(ml_dtypes.bfloat16).reshape(128, -1)

    im_base = dict(
        ohtbl=ohtbl,
        repq=repq, repx=repx,
        xT=np.ascontiguousarray(x.T),
        sidx=wrap16(s_idx), didx=wrap16(d_idx),
        dlfb=dlf.astype(ml_dtypes.bfloat16),
        mneg=maskneg,
        iota3=np.stack([np.ones(GCP), kk, kk ** 2]).astype(np.float32),
        triM=triM,
        oiT=oiT,
        qbias=qbias,
        iotaF=np.tile(np.arange(128, dtype=np.float32)[None, :], (128, 1)).astype(ml_dtypes.bfloat16),
    )
    for l in range(2):
        A = np.zeros((128, 8), np.float32)
        as_l = np.asarray(inputs[f"as{l}"], np.float32)
        ad_l = np.asarray(inputs[f"ad{l}"], np.float32)
        for h in range(HEADS):
            A[h * HID:(h + 1) * HID, h] = as_l[h]
            A[h * HID:(h + 1) * HID, 4 + h] = ad_l[h]
        im_base[f"W{l}"] = np.asarray(inputs[f"W{l}"], np.float32)
        im_base[f"Av{l}"] = A
        im_base[f"bcol{l}"] = np.asarray(inputs[f"b{l}"], np.float32).reshape(128, 1)
    in_maps = []
    for c in range(N_CORES):
        selm = np.zeros((128, DPC), np.float32)
        for k in range(DPC):
            selm[DPC * c + k, k] = 1.0
        in_maps.append({**im_base, "sel": selm})
    return in_maps


def kernel(**inputs) -> np.ndarray:
    from concourse.bass_utils import run_bass_kernel_spmd
    s_idx, d_idx, dlf, maskneg, chunk_tile = _edge_prep(np.asarray(inputs["edge_index"]))
    C = s_idx.shape[1]
    nc = _get_program(C, chunk_tile)
    in_maps = _host_inputs(inputs, s_idx, d_idx, dlf, maskneg, C)
    res = run_bass_kernel_spmd(nc, in_maps, list(range(N_CORES))).results
    return _assemble(inputs, res)


def _assemble(inputs, res):
    # kf[l, g, d, q]
    kf = np.zeros((3, G, 128, Q), np.float64)
    for c in range(N_CORES):
        s0t1 = np.asarray(res[c]["s0t1"], np.float64)    # [2Q, 192]
        S0, T1 = s0t1[0::2, :], s0t1[1::2, :]            # [Q, 192]
        mndl2 = np.asarray(res[c]["mndl2"], np.float64)  # [16, 24]
        mn = mndl2[:, 0::2].T.reshape(-1)                # [192] slot-ordered
        dl = mndl2[:, 1::2].T.reshape(-1)
        qvT = (mn[None, :] * S0 + dl[None, :] * T1) / (S0 + 1e-8)
        for u in range(UNITS):
            l, g = divmod(u, G)
            kf[l, g, DPC * c:DPC * (c + 1), :] = qvT[:, u * DPC:(u + 1) * DPC].T
    pmean = res[0]["pmean"] / 512.0          # [128, 12]
    pmax = res[0]["pmax"]
    pool_w = np.asarray(inputs["pool_w"], np.float64)
    beta = np.asarray(inputs["beta"], np.float64)
    h0 = float(np.asarray(inputs["h0"]).reshape(-1)[0])
    h_list, k_list = [], []
    for l in range(3):
        wp = (pool_w[0] * pmean[:, l * G:(l + 1) * G] + pool_w[1] * pmax[:, l * G:(l + 1) * G]).T
        lpW = np.asarray(inputs[f"lpW{l}"], np.float64)
        lpb = np.asarray(inputs[f"lpb{l}"], np.float64)
        h_list.append(wp @ lpW + lpb)
        kW = np.asarray(inputs[f"kW{l}"], np.float64)
        kb = np.asarray(inputs[f"kb{l}"], np.float64)
        k_list.append(kf[l].reshape(G, 128 * Q) @ kW + kb)
    main_out = np.mean(h_list, axis=0)
    kde_out = np.mean(k_list, axis=0)
    risk = (main_out + kde_out) @ beta + h0
    return risk.astype(np.float32)
